# revision 27
# baseline (speedup 1.0000x reference)
"""AdaptiveHyperbolicTripletLoss on 8 TRN2 NeuronCores (Bass/Tile).

Strategy (class-sharded data parallel):
  - 64 label classes; core i owns classes [8i, 8i+8). Anchors are grouped by
    class onto partitions: each class occupies ceil(m_c/A_p) whole partitions
    (A_p anchor slots per partition), so every SBUF partition holds anchors of
    exactly one class. Host computes the sharding permutation + its direct
    byproducts (per-class member tables, class sizes, rank-in-class) and the
    input-independent sampling uniforms (fixed jax PRNG key 42).
  - Device computes: sampling ranks from the uniforms (exact trunc semantics),
    positive index via per-partition table gather (gpsimd indirect_copy),
    negative index via monotone member-table counting (tensor_scalar+accum),
    embedding row gathers (gpsimd dma_gather from DRAM), row norms (computed
    locally + AllGather), Poincare distances, adaptive-margin triplet loss,
    masked reduction, and a final AllReduce of the partial sums.

Query layout per core: [128 partitions, F_tot = 5*A_p columns], query
(P, F=jj*5+k) <-> (anchor slot jj of partition P, triplet k). Gather slot
order n = F*128 + P, so dma_gather output [p, i, :] is directly [P, F] aligned
and the wrapped index position (n%16, n//16) = (P%16, 8F + P//16) is affine.
"""

import math
import numpy as np

import jax

_CPU = jax.devices("cpu")[0]

from concourse import bass, bacc, tile, mybir
from concourse import bass_utils

B, D, NCLS, K = 8192, 128, 64, 5
NCORES = 8
CLS_PER_CORE = NCLS // NCORES
MARGIN, BF, EPS = 1.0, 2.0, 1e-7
BIG = 20000.0
F32 = mybir.dt.float32
BF16 = mybir.dt.bfloat16
I16 = mybir.dt.int16
U16 = mybir.dt.uint16
I32 = mybir.dt.int32
ALU = mybir.AluOpType
ACTF = mybir.ActivationFunctionType


# ----------------------------------------------------------------------------
# host-side sharding prep
# ----------------------------------------------------------------------------

def _pick_layout(cnt):
    """Smallest A_p >= 10 such that every core's classes fit in 128 partitions."""
    for A_p in range(10, 65):
        ok = True
        for i in range(NCORES):
            parts = sum(int(math.ceil(max(int(cnt[c]), 1) / A_p))
                        for c in range(i * CLS_PER_CORE, (i + 1) * CLS_PER_CORE))
            if parts > 128:
                ok = False
                break
        if ok:
            return A_p
    raise ValueError("no layout fits")


def host_prep(labels_np):
    labels = np.asarray(labels_np).astype(np.int64).ravel()
    assert labels.shape[0] == B
    cnt = np.bincount(labels, minlength=NCLS)
    A_p = _pick_layout(cnt)
    FT = 5 * A_p
    W_s = int(np.ceil((max(int(cnt.max()), 1) + 1) / 8.0)) * 8  # member-table width

    skey = jax.random.key(42)
    kp, kn = jax.random.split(skey)
    with jax.default_device(_CPU):
        u_p = np.asarray(jax.random.uniform(kp, (B, K)), dtype=np.float32)
        u_n = np.asarray(jax.random.uniform(kn, (B, K)), dtype=np.float32)

    sel = [np.where(labels == c)[0] for c in range(NCLS)]

    cores = []
    for i in range(NCORES):
        u_pos = np.zeros((128, FT), np.float32)
        u_neg = np.zeros((128, FT), np.float32)
        poscnt = np.ones((128, FT), np.float32)
        negcnt = np.ones((128, FT), np.float32)
        p_bf = np.zeros((128, FT), np.float32)
        valid = np.zeros((128, FT), np.float32)
        sel_pp = np.zeros((128, W_s), np.int16)
        g_pp = np.full((128, W_s), BIG, np.float32)
        aidx = np.zeros(128 * A_p, np.int64)
        amask = np.zeros(128 * A_p, np.float32)
        cursor = 0
        for cl in range(CLS_PER_CORE):
            c = i * CLS_PER_CORE + cl
            mem = sel[c]
            m = len(mem)
            nparts = int(math.ceil(max(m, 1) / A_p))
            prows = slice(cursor, cursor + nparts)
            if m > 0:
                sel_pp[prows, :m] = mem.astype(np.int16)[None, :]
                g_pp[prows, :m] = (mem - np.arange(m)).astype(np.float32)[None, :]
            ok = 1.0 if (2 <= m < B) else 0.0
            for s in range(m):
                P = cursor + s // A_p
                jj = s % A_p
                aidx[P * A_p + jj] = mem[s]
                amask[P * A_p + jj] = 1.0
                F0 = jj * 5
                u_pos[P, F0:F0 + 5] = u_p[mem[s]]
                u_neg[P, F0:F0 + 5] = u_n[mem[s]]
                poscnt[P, F0:F0 + 5] = m - 1 if m > 1 else 1
                negcnt[P, F0:F0 + 5] = B - m
                p_bf[P, F0:F0 + 5] = s
                valid[P, F0:F0 + 5] = ok
            cursor += nparts
        assert cursor <= 128
        cores.append(dict(u_pos=u_pos, u_neg=u_neg, poscnt=poscnt, negcnt=negcnt,
                          p_bf=p_bf, valid=valid, sel_pp=sel_pp, g_pp=g_pp,
                          sel_f=sel_pp.copy(),
                          iota_f=np.tile(np.arange(W_s, dtype=np.int16), (128, 1)),
                          aidx=aidx, amask=amask))
    return cores, A_p, FT, W_s


# ----------------------------------------------------------------------------
# device program
# ----------------------------------------------------------------------------

def build(A_p, FT, W_s, debug_outs=False):
    nc = bacc.Bacc("TRN2", target_bir_lowering=False, debug=False,
                   num_devices=NCORES)
    emb_full = nc.declare_dram_parameter("emb_full", [B, D], F32, isOutput=False)
    emb_slice = nc.declare_dram_parameter("emb_slice", [B // NCORES, D], F32, isOutput=False)
    aemb = nc.declare_dram_parameter("aemb", [128 * A_p, D], F32, isOutput=False)
    d_u_pos = nc.declare_dram_parameter("u_pos", [128, FT], F32, isOutput=False)
    d_u_neg = nc.declare_dram_parameter("u_neg", [128, FT], F32, isOutput=False)
    d_poscnt = nc.declare_dram_parameter("poscnt", [128, FT], F32, isOutput=False)
    d_negcnt = nc.declare_dram_parameter("negcnt", [128, FT], F32, isOutput=False)
    d_p_bf = nc.declare_dram_parameter("p_bf", [128, FT], F32, isOutput=False)
    d_valid = nc.declare_dram_parameter("valid", [128, FT], F32, isOutput=False)
    d_sel_pp = nc.declare_dram_parameter("sel_pp", [128, W_s], I16, isOutput=False)
    d_g_pp = nc.declare_dram_parameter("g_pp", [128, W_s], F32, isOutput=False)
    d_sel_f = nc.declare_dram_parameter("sel_f", [128, W_s], I16, isOutput=False)
    d_iota = nc.declare_dram_parameter("iota_f", [128, W_s], I16, isOutput=False)
    out = nc.declare_dram_parameter("out", [1, 4], F32, isOutput=True)
    if debug_outs:
        dbg_pos = nc.declare_dram_parameter("dbg_pos", [128, FT], F32, isOutput=True)
        dbg_neg = nc.declare_dram_parameter("dbg_neg", [128, FT], F32, isOutput=True)
        dbg_dp = nc.declare_dram_parameter("dbg_dp", [128, FT], F32, isOutput=True)
        dbg_dn = nc.declare_dram_parameter("dbg_dn", [128, FT], F32, isOutput=True)

    RG = [list(range(NCORES))]
    NV = 16 * FT  # indirect_copy valid indices per 16-partition group

    with tile.TileContext(nc) as tc:
        with tc.tile_pool(name="main", bufs=1) as pool, \
             tc.tile_pool(name="dram", bufs=1, space="DRAM") as dram:

            # ---------------- load per-query constants & tables
            up = pool.tile([128, FT], F32); nc.sync.dma_start(out=up[:], in_=d_u_pos[:])
            un = pool.tile([128, FT], F32); nc.sync.dma_start(out=un[:], in_=d_u_neg[:])
            pc = pool.tile([128, FT], F32); nc.sync.dma_start(out=pc[:], in_=d_poscnt[:])
            ngc = pool.tile([128, FT], F32); nc.sync.dma_start(out=ngc[:], in_=d_negcnt[:])
            pbf = pool.tile([128, FT], F32); nc.sync.dma_start(out=pbf[:], in_=d_p_bf[:])
            vld = pool.tile([128, FT], F32); nc.sync.dma_start(out=vld[:], in_=d_valid[:])
            gpp = pool.tile([128, W_s], F32); nc.sync.dma_start(out=gpp[:], in_=d_g_pp[:])
            self_f = pool.tile([128, W_s], I16); nc.sync.dma_start(out=self_f[:], in_=d_sel_f[:])
            iota_f = pool.tile([128, W_s], I16); nc.sync.dma_start(out=iota_f[:], in_=d_iota[:])

            def exact_trunc_rank(u, cnt_t):
                """r = min(trunc(u*cnt), max(cnt-1,0)) with rounding-mode-proof trunc."""
                x = pool.tile([128, FT], F32, tag="rk_x")
                nc.vector.tensor_mul(x[:], u[:], cnt_t[:])
                ti = pool.tile([128, FT], I32, tag="rk_ti")
                nc.vector.tensor_copy(ti[:], x[:])
                tf = pool.tile([128, FT], F32, tag="rk_tf")
                nc.vector.tensor_copy(tf[:], ti[:])
                fx = pool.tile([128, FT], F32, tag="rk_fx")
                nc.vector.tensor_tensor(fx[:], tf[:], x[:], ALU.is_gt)
                r = pool.tile([128, FT], F32, tag="rk_r")
                nc.vector.tensor_tensor(r[:], tf[:], fx[:], ALU.subtract)
                cap = pool.tile([128, FT], F32, tag="rk_cap")
                nc.vector.tensor_scalar(cap[:], cnt_t[:], 1.0, None, ALU.subtract)
                nc.vector.tensor_scalar_max(cap[:], cap[:], 0.0)
                nc.vector.tensor_tensor(r[:], r[:], cap[:], ALU.min)
                return r

            # ---------------- shared helpers/tiles
            NI = 128 * FT

            def to_wrapped(slot_t, name):
                wrA = pool.tile([128, 8 * FT], I16, tag=name + "A")
                # wr[p0, 8F + a] = slot[16a + p0, F]; one DMA per group a
                for a in range(8):
                    nc.sync.dma_start(
                        out=wrA[0:16].rearrange("p (f a) -> p a f", a=8)[:, a, :],
                        in_=slot_t[16 * a:16 * (a + 1), :])
                for k in [16, 32, 64]:  # replicate idx block to all 8 gpsimd cores
                    nc.sync.dma_start(out=wrA[k:2 * k, :], in_=wrA[0:k, :])
                return wrA

            def gather_rows(wr_t, name):
                # chunked: SWDGE ring holds ~1024 descriptors
                g = pool.tile([128, FT, D], F32, tag=name)
                done = 0
                while done < NI:
                    n = min(1024, NI - done)
                    nc.gpsimd.dma_gather(
                        g[:, done // 128:(done + n) // 128, :], emb_full[:],
                        wr_t[:, done // 16:(done + n) // 16], n, n, D,
                        queue_num=0)
                    done += n
                return g

            # ---------------- positive index -> wrap -> gather (GpSimd early)
            rp = exact_trunc_rank(up, pc)
            geb = pool.tile([128, FT], F32)
            nc.vector.tensor_tensor(geb[:], rp[:], pbf[:], ALU.is_ge)
            rpp = pool.tile([128, FT], F32)
            nc.vector.tensor_tensor(rpp[:], rp[:], geb[:], ALU.add)
            # pos_idx[q] = sel[class(P), r'(q)] as an int16 masked sum
            rpp16 = pool.tile([128, FT], I16)
            nc.vector.tensor_copy(rpp16[:], rpp[:])
            mask3 = pool.tile([128, FT, W_s], I16)
            iota_e = iota_f[:].unsqueeze(1).broadcast_to((128, FT, W_s))
            rpp_e = rpp16[:].unsqueeze(2).broadcast_to((128, FT, W_s))
            sel_e = self_f[:].unsqueeze(1).broadcast_to((128, FT, W_s))
            nc.vector.tensor_tensor(mask3[:], iota_e, rpp_e, ALU.is_equal)
            nc.vector.tensor_tensor(mask3[:], mask3[:], sel_e, ALU.mult)
            posidx = pool.tile([128, FT], I16)
            with nc.allow_low_precision(reason="one-hot int16 sum, values < 2^13"):
                nc.vector.tensor_reduce(
                    posidx[:].rearrange("p (f o) -> p f o", o=1),
                    mask3[:], mybir.AxisListType.X, ALU.add)
            poswr = to_wrapped(posidx, "pw")
            posg = gather_rows(poswr, "posg")

            # ---------------- anchors (DVE, overlaps pos gather)
            at = pool.tile([128, A_p, D], F32)
            nc.sync.dma_start(out=at[:], in_=aemb[:].rearrange("(p t) d -> p t d", p=128))
            asq = pool.tile([128, A_p, D], F32)
            nc.vector.tensor_mul(asq[:], at[:], at[:])
            nx_a = pool.tile([128, A_p], F32)
            nc.vector.tensor_reduce(nx_a[:], asq[:], mybir.AxisListType.X, ALU.add)
            nx = nx_a[:].unsqueeze(2).broadcast_to((128, A_p, 5))  # 3D view
            a_exp = at[:].unsqueeze(2).broadcast_to((128, A_p, 5, D))

            def q3(t):  # [128, FT] tile -> [128, A_p, 5] view
                return t[:].rearrange("p (t k) -> p t k", t=A_p)

            # ---------------- negative index (DVE, overlaps pos gather)
            rn = exact_trunc_rank(un, ngc)
            tcnt = pool.tile([128, FT], F32)
            scratch = pool.tile([128, W_s], F32)
            for col in range(FT):
                nc.vector.tensor_scalar(
                    scratch[:], gpp[:], rn[:, col:col + 1], None, ALU.is_le,
                    ALU.add, accum_out=tcnt[:, col:col + 1])
            negidx_f = pool.tile([128, FT], F32)
            nc.vector.tensor_tensor(negidx_f[:], rn[:], tcnt[:], ALU.add)
            negidx = pool.tile([128, FT], I16)
            nc.vector.tensor_copy(negidx[:], negidx_f[:])
            negwr = to_wrapped(negidx, "nw")
            negg = gather_rows(negwr, "negg")

            if debug_outs:
                pf = pool.tile([128, FT], F32, tag="dbgc")
                nc.vector.tensor_copy(pf[:], posidx[:])
                nc.sync.dma_start(out=dbg_pos[:], in_=pf[:])
                nf = pool.tile([128, FT], F32, tag="dbgc2")
                nc.vector.tensor_copy(nf[:], negidx[:])
                nc.sync.dma_start(out=dbg_neg[:], in_=nf[:])

            # ---------------- Poincare distance per set (dist-pos overlaps
            # the neg gather descriptor generation)
            prod = pool.tile([128, A_p, 5, D], F32, tag="prod")
            dotv = pool.tile([128, FT], F32, tag="dotv")
            negone = pool.tile([128, 1], F32, tag="negone")
            nc.vector.memset(negone[:], -1.0)
            zerob = pool.tile([128, 1], F32, tag="zerob")
            nc.vector.memset(zerob[:], 0.0)
            nyv = pool.tile([128, FT], F32, tag="nyv")

            def distances(g_t, name):
                g3 = g_t[:].rearrange("p (t k) d -> p t k d", t=A_p)
                nc.vector.tensor_mul(prod[:], g3, g3)
                nc.vector.tensor_reduce(
                    nyv[:].rearrange("p (t k) -> p t k", t=A_p), prod[:],
                    mybir.AxisListType.X, ALU.add)
                ny_t = nyv
                nc.vector.tensor_mul(prod[:], g3, a_exp)
                nc.vector.tensor_reduce(
                    dotv[:].rearrange("p (t k) -> p t k", t=A_p), prod[:],
                    mybir.AxisListType.X, ALU.add)
                sq = pool.tile([128, FT], F32, tag=name + "sq")
                nc.vector.tensor_tensor(q3(sq), nx, q3(ny_t), ALU.add)
                m2d = pool.tile([128, FT], F32, tag="m2d")
                nc.vector.tensor_scalar(m2d[:], dotv[:], -2.0, None, ALU.mult)
                nc.vector.tensor_tensor(sq[:], sq[:], m2d[:], ALU.add)
                onx = pool.tile([128, FT], F32, tag="onx")
                nc.vector.tensor_scalar(q3(onx), nx, -1.0, 1.0, ALU.mult, ALU.add)
                ony = pool.tile([128, FT], F32, tag="ony")
                nc.vector.tensor_scalar(ony[:], ny_t[:], -1.0, 1.0, ALU.mult, ALU.add)
                den = pool.tile([128, FT], F32, tag="den")
                nc.vector.tensor_mul(den[:], onx[:], ony[:])
                nc.vector.tensor_scalar_max(den[:], den[:], EPS)
                rec = pool.tile([128, FT], F32, tag="rec")
                nc.vector.reciprocal(rec[:], den[:])
                arg = pool.tile([128, FT], F32, tag=name + "arg")
                nc.vector.tensor_mul(arg[:], sq[:], rec[:])
                nc.vector.tensor_scalar(arg[:], arg[:], 2.0, 1.0, ALU.mult, ALU.add)
                nc.vector.tensor_scalar_max(arg[:], arg[:], 1.0 + EPS)
                # arccosh(x) = ln(x + sqrt(x^2 - 1))
                s1 = pool.tile([128, FT], F32, tag="acs1")
                nc.scalar.activation(s1[:], arg[:], ACTF.Square, bias=zerob[:])
                nc.scalar.activation(s1[:], s1[:], ACTF.Sqrt, bias=negone[:])
                nc.vector.tensor_tensor(s1[:], s1[:], arg[:], ALU.add)
                dd = pool.tile([128, FT], F32, tag=name + "d")
                nc.scalar.activation(dd[:], s1[:], ACTF.Ln, bias=zerob[:])
                return dd

            d_pos = distances(posg, "dp")
            d_neg = distances(negg, "dn")
            if debug_outs:
                nc.sync.dma_start(out=dbg_dp[:], in_=d_pos[:])
                nc.sync.dma_start(out=dbg_dn[:], in_=d_neg[:])

            # ---------------- triplet + masked partial sums
            anrm = pool.tile([128, A_p], F32)
            nc.scalar.activation(anrm[:], nx_a[:], ACTF.Sqrt, bias=zerob[:])
            marg = pool.tile([128, A_p], F32)
            nc.vector.tensor_scalar(marg[:], anrm[:], 2.0 * MARGIN, MARGIN,
                                    ALU.mult, ALU.add)
            marg_exp = marg[:].unsqueeze(2).broadcast_to((128, A_p, 5))
            trip = pool.tile([128, FT], F32)
            nc.vector.tensor_tensor(trip[:], d_pos[:], d_neg[:], ALU.subtract)
            nc.vector.tensor_tensor(q3(trip), q3(trip), marg_exp, ALU.add)
            nc.vector.tensor_scalar_max(trip[:], trip[:], 0.0)
            losses = pool.tile([128, FT], F32)
            nc.vector.tensor_mul(losses[:], trip[:], vld[:])
            act = pool.tile([128, FT], F32)
            nc.vector.tensor_scalar(act[:], trip[:], 0.0, None, ALU.is_gt)
            nc.vector.tensor_mul(act[:], act[:], vld[:])

            part = pool.tile([128, 4], F32)
            nc.vector.tensor_reduce(part[:, 0:1], losses[:], mybir.AxisListType.X, ALU.add)
            nc.vector.tensor_reduce(part[:, 1:2], act[:], mybir.AxisListType.X, ALU.add)
            nc.vector.tensor_reduce(part[:, 2:3], vld[:], mybir.AxisListType.X, ALU.add)
            nc.vector.memset(part[:, 3:4], 0.0)
            psum1 = pool.tile([1, 4], F32)
            nc.gpsimd.tensor_reduce(psum1[:], part[:], mybir.AxisListType.C, ALU.add)

            # ---------------- AllReduce partials
            ar_in = dram.tile([1, 4], F32)
            nc.sync.dma_start(out=ar_in[:], in_=psum1[:])
            ar_out = dram.tile([1, 4], F32)
            nc.gpsimd.collective_compute(
                "AllReduce", ALU.add, replica_groups=RG,
                ins=[ar_in.opt()], outs=[ar_out.opt()])
            tot = pool.tile([1, 4], F32)
            nc.sync.dma_start(out=tot[:], in_=ar_out[:])

            # ---------------- finalize: [loss, num_active, total, ratio]
            den4 = pool.tile([1, 1], F32)
            nc.vector.tensor_scalar_max(den4[:], tot[:, 2:3], 1.0)
            rec4 = pool.tile([1, 1], F32)
            nc.vector.reciprocal(rec4[:], den4[:])
            res = pool.tile([1, 4], F32)
            nc.vector.tensor_scalar(res[:, 0:1], tot[:, 0:1], rec4[:], None, ALU.mult)
            nc.vector.tensor_copy(res[:, 1:2], tot[:, 1:2])
            nc.vector.tensor_copy(res[:, 2:3], tot[:, 2:3])
            nc.vector.tensor_scalar(res[:, 3:4], tot[:, 1:2], rec4[:], None, ALU.mult)
            nc.sync.dma_start(out=out[:], in_=res[:])

    nc.finalize()  # run bacc compile (regalloc etc.) before PJRT serialization
    return nc


# ----------------------------------------------------------------------------
# entry point
# ----------------------------------------------------------------------------

_CACHE = {}


def _get_nc(A_p, FT, W_s, debug_outs):
    key = (A_p, FT, W_s, debug_outs)
    if key not in _CACHE:
        _CACHE[key] = build(A_p, FT, W_s, debug_outs)
    return _CACHE[key]


def run(inputs, debug_outs=False, trace=False):
    emb = np.ascontiguousarray(np.asarray(inputs["embeddings"], dtype=np.float32))
    labels = inputs["labels"]
    cores, A_p, FT, W_s = host_prep(labels)
    nc = _get_nc(A_p, FT, W_s, debug_outs)

    in_maps = []
    for i in range(NCORES):
        c = cores[i]
        arows = emb[c["aidx"].astype(np.int64)] * c["amask"][:, None]
        in_maps.append({
            "emb_full": emb,
            "emb_slice": np.ascontiguousarray(emb[i * (B // NCORES):(i + 1) * (B // NCORES)]),
            "aemb": np.ascontiguousarray(arows.astype(np.float32)),
            "u_pos": c["u_pos"], "u_neg": c["u_neg"],
            "poscnt": c["poscnt"], "negcnt": c["negcnt"],
            "p_bf": c["p_bf"], "valid": c["valid"],
            "sel_pp": c["sel_pp"], "g_pp": c["g_pp"],
            "sel_f": c["sel_f"], "iota_f": c["iota_f"],
        })

    res = bass_utils.run_bass_kernel_spmd(
        nc, in_maps, core_ids=list(range(NCORES)), trace=trace)
    return res, cores, A_p, FT


def kernel(**inputs):
    res, _, _, _ = run(inputs, debug_outs=False, trace=False)
    o = np.asarray(res.results[0]["out"]).reshape(4)
    loss = np.float32(o[0])
    num_active = np.int32(round(float(o[1])))
    total = np.int32(round(float(o[2])))
    ratio = np.float32(o[3])
    return loss, num_active, total, ratio


# revision 29
# speedup vs baseline: 1.1270x; 1.1270x over previous
"""AdaptiveHyperbolicTripletLoss on 8 TRN2 NeuronCores (Bass/Tile).

Strategy (class-sharded data parallel):
  - 64 label classes; core i owns classes [8i, 8i+8). Anchors are grouped by
    class onto partitions: each class occupies ceil(m_c/A_p) whole partitions
    (A_p anchor slots per partition), so every SBUF partition holds anchors of
    exactly one class. Host computes the sharding permutation + its direct
    byproducts (per-class member tables, class sizes, rank-in-class) and the
    input-independent sampling uniforms (fixed jax PRNG key 42).
  - Device computes: sampling ranks from the uniforms (exact trunc semantics),
    positive index via per-partition table gather (gpsimd indirect_copy),
    negative index via monotone member-table counting (tensor_scalar+accum),
    embedding row gathers (gpsimd dma_gather from DRAM), row norms (computed
    locally + AllGather), Poincare distances, adaptive-margin triplet loss,
    masked reduction, and a final AllReduce of the partial sums.

Query layout per core: [128 partitions, F_tot = 5*A_p columns], query
(P, F=jj*5+k) <-> (anchor slot jj of partition P, triplet k). Gather slot
order n = F*128 + P, so dma_gather output [p, i, :] is directly [P, F] aligned
and the wrapped index position (n%16, n//16) = (P%16, 8F + P//16) is affine.
"""

import math
import numpy as np

import jax

_CPU = jax.devices("cpu")[0]

from concourse import bass, bacc, tile, mybir
from concourse import bass_utils

B, D, NCLS, K = 8192, 128, 64, 5
NCORES = 8
CLS_PER_CORE = NCLS // NCORES
MARGIN, BF, EPS = 1.0, 2.0, 1e-7
BIG = 20000.0
F32 = mybir.dt.float32
BF16 = mybir.dt.bfloat16
I16 = mybir.dt.int16
U16 = mybir.dt.uint16
I32 = mybir.dt.int32
ALU = mybir.AluOpType
ACTF = mybir.ActivationFunctionType


# ----------------------------------------------------------------------------
# host-side sharding prep
# ----------------------------------------------------------------------------

def _pick_layout(cnt):
    """Smallest A_p >= 10 such that every core's classes fit in 128 partitions."""
    for A_p in range(10, 65):
        ok = True
        for i in range(NCORES):
            parts = sum(int(math.ceil(max(int(cnt[c]), 1) / A_p))
                        for c in range(i * CLS_PER_CORE, (i + 1) * CLS_PER_CORE))
            if parts > 128:
                ok = False
                break
        if ok:
            return A_p
    raise ValueError("no layout fits")


def host_prep(labels_np):
    labels = np.asarray(labels_np).astype(np.int64).ravel()
    assert labels.shape[0] == B
    cnt = np.bincount(labels, minlength=NCLS)
    A_p = _pick_layout(cnt)
    FT = 5 * A_p
    W_s = int(np.ceil((max(int(cnt.max()), 1) + 1) / 8.0)) * 8  # member-table width

    skey = jax.random.key(42)
    kp, kn = jax.random.split(skey)
    with jax.default_device(_CPU):
        u_p = np.asarray(jax.random.uniform(kp, (B, K)), dtype=np.float32)
        u_n = np.asarray(jax.random.uniform(kn, (B, K)), dtype=np.float32)

    sel = [np.where(labels == c)[0] for c in range(NCLS)]

    cores = []
    for i in range(NCORES):
        u_pos = np.zeros((128, FT), np.float32)
        u_neg = np.zeros((128, FT), np.float32)
        poscnt = np.ones((128, FT), np.float32)
        negcnt = np.ones((128, FT), np.float32)
        p_bf = np.zeros((128, FT), np.float32)
        valid = np.zeros((128, FT), np.float32)
        sel_pp = np.zeros((128, W_s), np.int16)
        g_pp = np.full((128, W_s), BIG, np.float32)
        aidx = np.zeros(128 * A_p, np.int64)
        amask = np.zeros(128 * A_p, np.float32)
        cursor = 0
        for cl in range(CLS_PER_CORE):
            c = i * CLS_PER_CORE + cl
            mem = sel[c]
            m = len(mem)
            nparts = int(math.ceil(max(m, 1) / A_p))
            prows = slice(cursor, cursor + nparts)
            if m > 0:
                sel_pp[prows, :m] = mem.astype(np.int16)[None, :]
                g_pp[prows, :m] = (mem - np.arange(m)).astype(np.float32)[None, :]
            ok = 1.0 if (2 <= m < B) else 0.0
            for s in range(m):
                P = cursor + s // A_p
                jj = s % A_p
                aidx[P * A_p + jj] = mem[s]
                amask[P * A_p + jj] = 1.0
                F0 = jj * 5
                u_pos[P, F0:F0 + 5] = u_p[mem[s]]
                u_neg[P, F0:F0 + 5] = u_n[mem[s]]
                poscnt[P, F0:F0 + 5] = m - 1 if m > 1 else 1
                negcnt[P, F0:F0 + 5] = B - m
                p_bf[P, F0:F0 + 5] = s
                valid[P, F0:F0 + 5] = ok
            cursor += nparts
        assert cursor <= 128
        cores.append(dict(u_pos=u_pos, u_neg=u_neg, poscnt=poscnt, negcnt=negcnt,
                          p_bf=p_bf, valid=valid, sel_pp=sel_pp, g_pp=g_pp,
                          sel_f=sel_pp.copy(),
                          iota_f=np.tile(np.arange(W_s, dtype=np.int16), (128, 1)),
                          aidx=aidx, amask=amask))
    return cores, A_p, FT, W_s


# ----------------------------------------------------------------------------
# device program
# ----------------------------------------------------------------------------

def build(A_p, FT, W_s, debug_outs=False):
    nc = bacc.Bacc("TRN2", target_bir_lowering=False, debug=False,
                   num_devices=NCORES)
    emb_full = nc.declare_dram_parameter("emb_full", [B, D], F32, isOutput=False)
    emb_slice = nc.declare_dram_parameter("emb_slice", [B // NCORES, D], F32, isOutput=False)
    aemb = nc.declare_dram_parameter("aemb", [128 * A_p, D], F32, isOutput=False)
    d_u_pos = nc.declare_dram_parameter("u_pos", [128, FT], F32, isOutput=False)
    d_u_neg = nc.declare_dram_parameter("u_neg", [128, FT], F32, isOutput=False)
    d_poscnt = nc.declare_dram_parameter("poscnt", [128, FT], F32, isOutput=False)
    d_negcnt = nc.declare_dram_parameter("negcnt", [128, FT], F32, isOutput=False)
    d_p_bf = nc.declare_dram_parameter("p_bf", [128, FT], F32, isOutput=False)
    d_valid = nc.declare_dram_parameter("valid", [128, FT], F32, isOutput=False)
    d_sel_pp = nc.declare_dram_parameter("sel_pp", [128, W_s], I16, isOutput=False)
    d_g_pp = nc.declare_dram_parameter("g_pp", [128, W_s], F32, isOutput=False)
    d_sel_f = nc.declare_dram_parameter("sel_f", [128, W_s], I16, isOutput=False)
    d_iota = nc.declare_dram_parameter("iota_f", [128, W_s], I16, isOutput=False)
    out = nc.declare_dram_parameter("out", [1, 4], F32, isOutput=True)
    if debug_outs:
        dbg_pos = nc.declare_dram_parameter("dbg_pos", [128, FT], F32, isOutput=True)
        dbg_neg = nc.declare_dram_parameter("dbg_neg", [128, FT], F32, isOutput=True)
        dbg_dp = nc.declare_dram_parameter("dbg_dp", [128, FT], F32, isOutput=True)
        dbg_dn = nc.declare_dram_parameter("dbg_dn", [128, FT], F32, isOutput=True)

    RG = [list(range(NCORES))]
    NV = 16 * FT  # indirect_copy valid indices per 16-partition group

    with tile.TileContext(nc) as tc:
        with tc.tile_pool(name="main", bufs=1) as pool, \
             tc.tile_pool(name="dram", bufs=1, space="DRAM") as dram:

            # ---------------- load per-query constants & tables
            up = pool.tile([128, FT], F32); nc.sync.dma_start(out=up[:], in_=d_u_pos[:])
            un = pool.tile([128, FT], F32); nc.sync.dma_start(out=un[:], in_=d_u_neg[:])
            pc = pool.tile([128, FT], F32); nc.sync.dma_start(out=pc[:], in_=d_poscnt[:])
            ngc = pool.tile([128, FT], F32); nc.sync.dma_start(out=ngc[:], in_=d_negcnt[:])
            pbf = pool.tile([128, FT], F32); nc.sync.dma_start(out=pbf[:], in_=d_p_bf[:])
            vld = pool.tile([128, FT], F32); nc.sync.dma_start(out=vld[:], in_=d_valid[:])
            gpp = pool.tile([128, W_s], F32); nc.sync.dma_start(out=gpp[:], in_=d_g_pp[:])
            self_f = pool.tile([128, W_s], I16); nc.sync.dma_start(out=self_f[:], in_=d_sel_f[:])
            iota_f = pool.tile([128, W_s], I16); nc.sync.dma_start(out=iota_f[:], in_=d_iota[:])

            def exact_trunc_rank(u, cnt_t):
                """r = min(trunc(u*cnt), max(cnt-1,0)) with rounding-mode-proof trunc."""
                x = pool.tile([128, FT], F32, tag="rk_x")
                nc.vector.tensor_mul(x[:], u[:], cnt_t[:])
                ti = pool.tile([128, FT], I32, tag="rk_ti")
                nc.vector.tensor_copy(ti[:], x[:])
                tf = pool.tile([128, FT], F32, tag="rk_tf")
                nc.vector.tensor_copy(tf[:], ti[:])
                fx = pool.tile([128, FT], F32, tag="rk_fx")
                nc.vector.tensor_tensor(fx[:], tf[:], x[:], ALU.is_gt)
                r = pool.tile([128, FT], F32, tag="rk_r")
                nc.vector.tensor_tensor(r[:], tf[:], fx[:], ALU.subtract)
                cap = pool.tile([128, FT], F32, tag="rk_cap")
                nc.vector.tensor_scalar(cap[:], cnt_t[:], 1.0, None, ALU.subtract)
                nc.vector.tensor_scalar_max(cap[:], cap[:], 0.0)
                nc.vector.tensor_tensor(r[:], r[:], cap[:], ALU.min)
                return r

            # ---------------- shared helpers/tiles
            NI = 128 * FT

            def to_wrapped(slot_t, name):
                wrA = pool.tile([128, 8 * FT], I16, tag=name + "A")
                # wr[p0, 8F + a] = slot[16a + p0, F]; one DMA per group a
                for a in range(8):
                    nc.sync.dma_start(
                        out=wrA[0:16].rearrange("p (f a) -> p a f", a=8)[:, a, :],
                        in_=slot_t[16 * a:16 * (a + 1), :])
                for k in [16, 32, 64]:  # replicate idx block to all 8 gpsimd cores
                    nc.sync.dma_start(out=wrA[k:2 * k, :], in_=wrA[0:k, :])
                return wrA

            def gather_rows(wr_t, name):
                # chunked: SWDGE ring holds ~1024 descriptors
                g = pool.tile([128, FT, D], F32, tag=name)
                done = 0
                while done < NI:
                    n = min(1024, NI - done)
                    nc.gpsimd.dma_gather(
                        g[:, done // 128:(done + n) // 128, :], emb_full[:],
                        wr_t[:, done // 16:(done + n) // 16], n, n, D,
                        queue_num=0)
                    done += n
                return g

            # ---------------- positive index -> wrap -> gather (GpSimd early)
            rp = exact_trunc_rank(up, pc)
            geb = pool.tile([128, FT], F32)
            nc.vector.tensor_tensor(geb[:], rp[:], pbf[:], ALU.is_ge)
            rpp = pool.tile([128, FT], F32)
            nc.vector.tensor_tensor(rpp[:], rp[:], geb[:], ALU.add)
            # pos_idx[q] = sel[class(P), r'(q)] as an int16 masked sum
            rpp16 = pool.tile([128, FT], I16)
            nc.vector.tensor_copy(rpp16[:], rpp[:])
            mask3 = pool.tile([128, FT, W_s], I16)
            iota_e = iota_f[:].unsqueeze(1).broadcast_to((128, FT, W_s))
            rpp_e = rpp16[:].unsqueeze(2).broadcast_to((128, FT, W_s))
            sel_e = self_f[:].unsqueeze(1).broadcast_to((128, FT, W_s))
            nc.vector.tensor_tensor(mask3[:], iota_e, rpp_e, ALU.is_equal)
            nc.vector.tensor_tensor(mask3[:], mask3[:], sel_e, ALU.mult)
            posidx = pool.tile([128, FT], I16)
            with nc.allow_low_precision(reason="one-hot int16 sum, values < 2^13"):
                nc.vector.tensor_reduce(
                    posidx[:].rearrange("p (f o) -> p f o", o=1),
                    mask3[:], mybir.AxisListType.X, ALU.add)
            # ---------------- anchors (DVE, overlaps pos gather)
            at = pool.tile([128, A_p, D], F32)
            nc.sync.dma_start(out=at[:], in_=aemb[:].rearrange("(p t) d -> p t d", p=128))
            asq = pool.tile([128, A_p, D], F32)
            nc.vector.tensor_mul(asq[:], at[:], at[:])
            nx_a = pool.tile([128, A_p], F32)
            nc.vector.tensor_reduce(nx_a[:], asq[:], mybir.AxisListType.X, ALU.add)
            nx = nx_a[:].unsqueeze(2).broadcast_to((128, A_p, 5))  # 3D view
            a_exp = at[:].unsqueeze(2).broadcast_to((128, A_p, 5, D))

            def q3(t):  # [128, FT] tile -> [128, A_p, 5] view
                return t[:].rearrange("p (t k) -> p t k", t=A_p)

            # ---------------- negative index (DVE, overlaps pos gather)
            rn = exact_trunc_rank(un, ngc)
            tcnt = pool.tile([128, FT], F32)
            scratch = pool.tile([128, W_s], F32)
            for col in range(FT):
                nc.vector.tensor_scalar(
                    scratch[:], gpp[:], rn[:, col:col + 1], None, ALU.is_le,
                    ALU.add, accum_out=tcnt[:, col:col + 1])
            negidx_f = pool.tile([128, FT], F32)
            nc.vector.tensor_tensor(negidx_f[:], rn[:], tcnt[:], ALU.add)
            negidx = pool.tile([128, FT], I16)
            nc.vector.tensor_copy(negidx[:], negidx_f[:])
            poswr = to_wrapped(posidx, "pw")
            posg = gather_rows(poswr, "posg")

            negwr = to_wrapped(negidx, "nw")
            negg = gather_rows(negwr, "negg")

            if debug_outs:
                pf = pool.tile([128, FT], F32, tag="dbgc")
                nc.vector.tensor_copy(pf[:], posidx[:])
                nc.sync.dma_start(out=dbg_pos[:], in_=pf[:])
                nf = pool.tile([128, FT], F32, tag="dbgc2")
                nc.vector.tensor_copy(nf[:], negidx[:])
                nc.sync.dma_start(out=dbg_neg[:], in_=nf[:])

            # ---------------- Poincare distance per set (dist-pos overlaps
            # the neg gather descriptor generation)
            prod = pool.tile([128, A_p, 5, D], F32, tag="prod")
            dotv = pool.tile([128, FT], F32, tag="dotv")
            negone = pool.tile([128, 1], F32, tag="negone")
            nc.vector.memset(negone[:], -1.0)
            zerob = pool.tile([128, 1], F32, tag="zerob")
            nc.vector.memset(zerob[:], 0.0)
            nyv = pool.tile([128, FT], F32, tag="nyv")

            def distances(g_t, name):
                g3 = g_t[:].rearrange("p (t k) d -> p t k d", t=A_p)
                nc.vector.tensor_mul(prod[:], g3, g3)
                nc.vector.tensor_reduce(
                    nyv[:].rearrange("p (t k) -> p t k", t=A_p), prod[:],
                    mybir.AxisListType.X, ALU.add)
                ny_t = nyv
                nc.vector.tensor_mul(prod[:], g3, a_exp)
                nc.vector.tensor_reduce(
                    dotv[:].rearrange("p (t k) -> p t k", t=A_p), prod[:],
                    mybir.AxisListType.X, ALU.add)
                sq = pool.tile([128, FT], F32, tag=name + "sq")
                nc.vector.tensor_tensor(q3(sq), nx, q3(ny_t), ALU.add)
                m2d = pool.tile([128, FT], F32, tag="m2d")
                nc.vector.tensor_scalar(m2d[:], dotv[:], -2.0, None, ALU.mult)
                nc.vector.tensor_tensor(sq[:], sq[:], m2d[:], ALU.add)
                onx = pool.tile([128, FT], F32, tag="onx")
                nc.vector.tensor_scalar(q3(onx), nx, -1.0, 1.0, ALU.mult, ALU.add)
                ony = pool.tile([128, FT], F32, tag="ony")
                nc.vector.tensor_scalar(ony[:], ny_t[:], -1.0, 1.0, ALU.mult, ALU.add)
                den = pool.tile([128, FT], F32, tag="den")
                nc.vector.tensor_mul(den[:], onx[:], ony[:])
                nc.vector.tensor_scalar_max(den[:], den[:], EPS)
                rec = pool.tile([128, FT], F32, tag="rec")
                nc.vector.reciprocal(rec[:], den[:])
                arg = pool.tile([128, FT], F32, tag=name + "arg")
                nc.vector.tensor_mul(arg[:], sq[:], rec[:])
                nc.vector.tensor_scalar(arg[:], arg[:], 2.0, 1.0, ALU.mult, ALU.add)
                nc.vector.tensor_scalar_max(arg[:], arg[:], 1.0 + EPS)
                # arccosh(x) = ln(x + sqrt(x^2 - 1))
                s1 = pool.tile([128, FT], F32, tag="acs1")
                nc.scalar.activation(s1[:], arg[:], ACTF.Square, bias=zerob[:])
                nc.scalar.activation(s1[:], s1[:], ACTF.Sqrt, bias=negone[:])
                nc.vector.tensor_tensor(s1[:], s1[:], arg[:], ALU.add)
                dd = pool.tile([128, FT], F32, tag=name + "d")
                nc.scalar.activation(dd[:], s1[:], ACTF.Ln, bias=zerob[:])
                return dd

            d_pos = distances(posg, "dp")
            d_neg = distances(negg, "dn")
            if debug_outs:
                nc.sync.dma_start(out=dbg_dp[:], in_=d_pos[:])
                nc.sync.dma_start(out=dbg_dn[:], in_=d_neg[:])

            # ---------------- triplet + masked partial sums
            anrm = pool.tile([128, A_p], F32)
            nc.scalar.activation(anrm[:], nx_a[:], ACTF.Sqrt, bias=zerob[:])
            marg = pool.tile([128, A_p], F32)
            nc.vector.tensor_scalar(marg[:], anrm[:], 2.0 * MARGIN, MARGIN,
                                    ALU.mult, ALU.add)
            marg_exp = marg[:].unsqueeze(2).broadcast_to((128, A_p, 5))
            trip = pool.tile([128, FT], F32)
            nc.vector.tensor_tensor(trip[:], d_pos[:], d_neg[:], ALU.subtract)
            nc.vector.tensor_tensor(q3(trip), q3(trip), marg_exp, ALU.add)
            nc.vector.tensor_scalar_max(trip[:], trip[:], 0.0)
            losses = pool.tile([128, FT], F32)
            nc.vector.tensor_mul(losses[:], trip[:], vld[:])
            act = pool.tile([128, FT], F32)
            nc.vector.tensor_scalar(act[:], trip[:], 0.0, None, ALU.is_gt)
            nc.vector.tensor_mul(act[:], act[:], vld[:])

            part = pool.tile([128, 4], F32)
            nc.vector.tensor_reduce(part[:, 0:1], losses[:], mybir.AxisListType.X, ALU.add)
            nc.vector.tensor_reduce(part[:, 1:2], act[:], mybir.AxisListType.X, ALU.add)
            nc.vector.tensor_reduce(part[:, 2:3], vld[:], mybir.AxisListType.X, ALU.add)
            nc.vector.memset(part[:, 3:4], 0.0)
            psum1 = pool.tile([1, 4], F32)
            nc.gpsimd.tensor_reduce(psum1[:], part[:], mybir.AxisListType.C, ALU.add)

            # ---------------- AllReduce partials
            ar_in = dram.tile([1, 4], F32)
            nc.sync.dma_start(out=ar_in[:], in_=psum1[:])
            ar_out = dram.tile([1, 4], F32)
            nc.gpsimd.collective_compute(
                "AllReduce", ALU.add, replica_groups=RG,
                ins=[ar_in.opt()], outs=[ar_out.opt()])
            tot = pool.tile([1, 4], F32)
            nc.sync.dma_start(out=tot[:], in_=ar_out[:])

            # ---------------- finalize: [loss, num_active, total, ratio]
            den4 = pool.tile([1, 1], F32)
            nc.vector.tensor_scalar_max(den4[:], tot[:, 2:3], 1.0)
            rec4 = pool.tile([1, 1], F32)
            nc.vector.reciprocal(rec4[:], den4[:])
            res = pool.tile([1, 4], F32)
            nc.vector.tensor_scalar(res[:, 0:1], tot[:, 0:1], rec4[:], None, ALU.mult)
            nc.vector.tensor_copy(res[:, 1:2], tot[:, 1:2])
            nc.vector.tensor_copy(res[:, 2:3], tot[:, 2:3])
            nc.vector.tensor_scalar(res[:, 3:4], tot[:, 1:2], rec4[:], None, ALU.mult)
            nc.sync.dma_start(out=out[:], in_=res[:])

    nc.finalize()  # run bacc compile (regalloc etc.) before PJRT serialization
    return nc


# ----------------------------------------------------------------------------
# entry point
# ----------------------------------------------------------------------------

_CACHE = {}


def _get_nc(A_p, FT, W_s, debug_outs):
    key = (A_p, FT, W_s, debug_outs)
    if key not in _CACHE:
        _CACHE[key] = build(A_p, FT, W_s, debug_outs)
    return _CACHE[key]


def run(inputs, debug_outs=False, trace=False):
    emb = np.ascontiguousarray(np.asarray(inputs["embeddings"], dtype=np.float32))
    labels = inputs["labels"]
    cores, A_p, FT, W_s = host_prep(labels)
    nc = _get_nc(A_p, FT, W_s, debug_outs)

    in_maps = []
    for i in range(NCORES):
        c = cores[i]
        arows = emb[c["aidx"].astype(np.int64)] * c["amask"][:, None]
        in_maps.append({
            "emb_full": emb,
            "emb_slice": np.ascontiguousarray(emb[i * (B // NCORES):(i + 1) * (B // NCORES)]),
            "aemb": np.ascontiguousarray(arows.astype(np.float32)),
            "u_pos": c["u_pos"], "u_neg": c["u_neg"],
            "poscnt": c["poscnt"], "negcnt": c["negcnt"],
            "p_bf": c["p_bf"], "valid": c["valid"],
            "sel_pp": c["sel_pp"], "g_pp": c["g_pp"],
            "sel_f": c["sel_f"], "iota_f": c["iota_f"],
        })

    res = bass_utils.run_bass_kernel_spmd(
        nc, in_maps, core_ids=list(range(NCORES)), trace=trace)
    return res, cores, A_p, FT


def kernel(**inputs):
    res, _, _, _ = run(inputs, debug_outs=False, trace=False)
    o = np.asarray(res.results[0]["out"]).reshape(4)
    loss = np.float32(o[0])
    num_active = np.int32(round(float(o[1])))
    total = np.int32(round(float(o[2])))
    ratio = np.float32(o[3])
    return loss, num_active, total, ratio


# revision 31
# speedup vs baseline: 1.2602x; 1.1182x over previous
"""AdaptiveHyperbolicTripletLoss on 8 TRN2 NeuronCores (Bass/Tile).

Strategy (class-sharded data parallel):
  - 64 label classes; core i owns classes [8i, 8i+8). Anchors are grouped by
    class onto partitions: each class occupies ceil(m_c/A_p) whole partitions
    (A_p anchor slots per partition), so every SBUF partition holds anchors of
    exactly one class. Host computes the sharding permutation + its direct
    byproducts (per-class member tables, class sizes, rank-in-class) and the
    input-independent sampling uniforms (fixed jax PRNG key 42).
  - Device computes: sampling ranks from the uniforms (exact trunc semantics),
    positive index via per-partition table gather (gpsimd indirect_copy),
    negative index via monotone member-table counting (tensor_scalar+accum),
    embedding row gathers (gpsimd dma_gather from DRAM), row norms (computed
    locally + AllGather), Poincare distances, adaptive-margin triplet loss,
    masked reduction, and a final AllReduce of the partial sums.

Query layout per core: [128 partitions, F_tot = 5*A_p columns], query
(P, F=jj*5+k) <-> (anchor slot jj of partition P, triplet k). Gather slot
order n = F*128 + P, so dma_gather output [p, i, :] is directly [P, F] aligned
and the wrapped index position (n%16, n//16) = (P%16, 8F + P//16) is affine.
"""

import math
import numpy as np

import jax

_CPU = jax.devices("cpu")[0]

from concourse import bass, bacc, tile, mybir
from concourse import bass_utils

B, D, NCLS, K = 8192, 128, 64, 5
NCORES = 8
CLS_PER_CORE = NCLS // NCORES
MARGIN, BF, EPS = 1.0, 2.0, 1e-7
BIG = 20000.0
F32 = mybir.dt.float32
BF16 = mybir.dt.bfloat16
I16 = mybir.dt.int16
U16 = mybir.dt.uint16
I32 = mybir.dt.int32
ALU = mybir.AluOpType
ACTF = mybir.ActivationFunctionType


# ----------------------------------------------------------------------------
# host-side sharding prep
# ----------------------------------------------------------------------------

def _pick_layout(cnt):
    """Smallest A_p >= 9 such that every core's classes fit in 128 partitions."""
    for A_p in range(9, 65):
        ok = True
        for i in range(NCORES):
            parts = sum(int(math.ceil(max(int(cnt[c]), 1) / A_p))
                        for c in range(i * CLS_PER_CORE, (i + 1) * CLS_PER_CORE))
            if parts > 128:
                ok = False
                break
        if ok:
            return A_p
    raise ValueError("no layout fits")


def host_prep(labels_np):
    labels = np.asarray(labels_np).astype(np.int64).ravel()
    assert labels.shape[0] == B
    cnt = np.bincount(labels, minlength=NCLS)
    A_p = _pick_layout(cnt)
    FT = 5 * A_p
    W_s = int(np.ceil((max(int(cnt.max()), 1) + 1) / 8.0)) * 8  # member-table width

    skey = jax.random.key(42)
    kp, kn = jax.random.split(skey)
    with jax.default_device(_CPU):
        u_p = np.asarray(jax.random.uniform(kp, (B, K)), dtype=np.float32)
        u_n = np.asarray(jax.random.uniform(kn, (B, K)), dtype=np.float32)

    sel = [np.where(labels == c)[0] for c in range(NCLS)]

    cores = []
    for i in range(NCORES):
        u_pos = np.zeros((128, FT), np.float32)
        u_neg = np.zeros((128, FT), np.float32)
        poscnt = np.ones((128, FT), np.float32)
        negcnt = np.ones((128, FT), np.float32)
        p_bf = np.zeros((128, FT), np.float32)
        valid = np.zeros((128, FT), np.float32)
        sel_pp = np.zeros((128, W_s), np.int16)
        g_pp = np.full((128, W_s), BIG, np.float32)
        aidx = np.zeros(128 * A_p, np.int64)
        amask = np.zeros(128 * A_p, np.float32)
        cursor = 0
        for cl in range(CLS_PER_CORE):
            c = i * CLS_PER_CORE + cl
            mem = sel[c]
            m = len(mem)
            nparts = int(math.ceil(max(m, 1) / A_p))
            prows = slice(cursor, cursor + nparts)
            if m > 0:
                sel_pp[prows, :m] = mem.astype(np.int16)[None, :]
                g_pp[prows, :m] = (mem - np.arange(m)).astype(np.float32)[None, :]
            ok = 1.0 if (2 <= m < B) else 0.0
            for s in range(m):
                P = cursor + s // A_p
                jj = s % A_p
                aidx[P * A_p + jj] = mem[s]
                amask[P * A_p + jj] = 1.0
                F0 = jj * 5
                u_pos[P, F0:F0 + 5] = u_p[mem[s]]
                u_neg[P, F0:F0 + 5] = u_n[mem[s]]
                poscnt[P, F0:F0 + 5] = m - 1 if m > 1 else 1
                negcnt[P, F0:F0 + 5] = B - m
                p_bf[P, F0:F0 + 5] = s
                valid[P, F0:F0 + 5] = ok
            cursor += nparts
        assert cursor <= 128
        cores.append(dict(u_pos=u_pos, u_neg=u_neg, poscnt=poscnt, negcnt=negcnt,
                          p_bf=p_bf, valid=valid, sel_pp=sel_pp, g_pp=g_pp,
                          sel_f=sel_pp.copy(),
                          iota_f=np.tile(np.arange(W_s, dtype=np.int16), (128, 1)),
                          aidx=aidx, amask=amask))
    return cores, A_p, FT, W_s


# ----------------------------------------------------------------------------
# device program
# ----------------------------------------------------------------------------

def build(A_p, FT, W_s, debug_outs=False):
    nc = bacc.Bacc("TRN2", target_bir_lowering=False, debug=False,
                   num_devices=NCORES)
    emb_full = nc.declare_dram_parameter("emb_full", [B, D], F32, isOutput=False)
    emb_slice = nc.declare_dram_parameter("emb_slice", [B // NCORES, D], F32, isOutput=False)
    aemb = nc.declare_dram_parameter("aemb", [128 * A_p, D], F32, isOutput=False)
    d_u_pos = nc.declare_dram_parameter("u_pos", [128, FT], F32, isOutput=False)
    d_u_neg = nc.declare_dram_parameter("u_neg", [128, FT], F32, isOutput=False)
    d_poscnt = nc.declare_dram_parameter("poscnt", [128, FT], F32, isOutput=False)
    d_negcnt = nc.declare_dram_parameter("negcnt", [128, FT], F32, isOutput=False)
    d_p_bf = nc.declare_dram_parameter("p_bf", [128, FT], F32, isOutput=False)
    d_valid = nc.declare_dram_parameter("valid", [128, FT], F32, isOutput=False)
    d_sel_pp = nc.declare_dram_parameter("sel_pp", [128, W_s], I16, isOutput=False)
    d_g_pp = nc.declare_dram_parameter("g_pp", [128, W_s], F32, isOutput=False)
    d_sel_f = nc.declare_dram_parameter("sel_f", [128, W_s], I16, isOutput=False)
    d_iota = nc.declare_dram_parameter("iota_f", [128, W_s], I16, isOutput=False)
    out = nc.declare_dram_parameter("out", [1, 4], F32, isOutput=True)
    if debug_outs:
        dbg_pos = nc.declare_dram_parameter("dbg_pos", [128, FT], F32, isOutput=True)
        dbg_neg = nc.declare_dram_parameter("dbg_neg", [128, FT], F32, isOutput=True)
        dbg_dp = nc.declare_dram_parameter("dbg_dp", [128, FT], F32, isOutput=True)
        dbg_dn = nc.declare_dram_parameter("dbg_dn", [128, FT], F32, isOutput=True)

    RG = [list(range(NCORES))]
    NV = 16 * FT  # indirect_copy valid indices per 16-partition group

    with tile.TileContext(nc) as tc:
        with tc.tile_pool(name="main", bufs=1) as pool, \
             tc.tile_pool(name="dram", bufs=1, space="DRAM") as dram:

            # ---------------- load per-query constants & tables
            up = pool.tile([128, FT], F32); nc.sync.dma_start(out=up[:], in_=d_u_pos[:])
            un = pool.tile([128, FT], F32); nc.sync.dma_start(out=un[:], in_=d_u_neg[:])
            pc = pool.tile([128, FT], F32); nc.sync.dma_start(out=pc[:], in_=d_poscnt[:])
            ngc = pool.tile([128, FT], F32); nc.sync.dma_start(out=ngc[:], in_=d_negcnt[:])
            pbf = pool.tile([128, FT], F32); nc.sync.dma_start(out=pbf[:], in_=d_p_bf[:])
            vld = pool.tile([128, FT], F32); nc.sync.dma_start(out=vld[:], in_=d_valid[:])
            gpp = pool.tile([128, W_s], F32); nc.sync.dma_start(out=gpp[:], in_=d_g_pp[:])
            self_f = pool.tile([128, W_s], I16); nc.sync.dma_start(out=self_f[:], in_=d_sel_f[:])
            iota_f = pool.tile([128, W_s], I16); nc.sync.dma_start(out=iota_f[:], in_=d_iota[:])

            def exact_trunc_rank(u, cnt_t):
                """r = min(trunc(u*cnt), max(cnt-1,0)) with rounding-mode-proof trunc."""
                x = pool.tile([128, FT], F32, tag="rk_x")
                nc.vector.tensor_mul(x[:], u[:], cnt_t[:])
                ti = pool.tile([128, FT], I32, tag="rk_ti")
                nc.vector.tensor_copy(ti[:], x[:])
                tf = pool.tile([128, FT], F32, tag="rk_tf")
                nc.vector.tensor_copy(tf[:], ti[:])
                fx = pool.tile([128, FT], F32, tag="rk_fx")
                nc.vector.tensor_tensor(fx[:], tf[:], x[:], ALU.is_gt)
                r = pool.tile([128, FT], F32, tag="rk_r")
                nc.vector.tensor_tensor(r[:], tf[:], fx[:], ALU.subtract)
                cap = pool.tile([128, FT], F32, tag="rk_cap")
                nc.vector.tensor_scalar(cap[:], cnt_t[:], 1.0, None, ALU.subtract)
                nc.vector.tensor_scalar_max(cap[:], cap[:], 0.0)
                nc.vector.tensor_tensor(r[:], r[:], cap[:], ALU.min)
                return r

            # constants + ACT table warmup (tables load during idle prologue)
            negone = pool.tile([128, 1], F32, tag="negone")
            nc.vector.memset(negone[:], -1.0)
            zerob = pool.tile([128, 1], F32, tag="zerob")
            nc.vector.memset(zerob[:], 0.0)
            warm = pool.tile([128, 1], F32, tag="warm")
            nc.scalar.activation(warm[:], zerob[:], ACTF.Square, bias=zerob[:])
            nc.scalar.activation(warm[:], zerob[:], ACTF.Sqrt, bias=zerob[:])
            nc.scalar.activation(warm[:], warm[:], ACTF.Ln, bias=negone[:])

            # ---------------- shared helpers/tiles
            NI = 128 * FT

            def to_wrapped(slot_t, name):
                wrA = pool.tile([128, 8 * FT], I16, tag=name + "A")
                # wr[p0, 8F + a] = slot[16a + p0, F]; one DMA per group a
                for a in range(8):
                    eng = nc.sync if a % 2 == 0 else nc.scalar
                    eng.dma_start(
                        out=wrA[0:16].rearrange("p (f a) -> p a f", a=8)[:, a, :],
                        in_=slot_t[16 * a:16 * (a + 1), :])
                for k in [16, 32, 64]:  # replicate idx block to all 8 gpsimd cores
                    nc.sync.dma_start(out=wrA[k:2 * k, :], in_=wrA[0:k, :])
                return wrA

            def gather_rows(wr_t, name):
                # chunked: SWDGE ring holds ~1024 descriptors
                g = pool.tile([128, FT, D], F32, tag=name)
                done = 0
                while done < NI:
                    n = min(1024, NI - done)
                    nc.gpsimd.dma_gather(
                        g[:, done // 128:(done + n) // 128, :], emb_full[:],
                        wr_t[:, done // 16:(done + n) // 16], n, n, D,
                        queue_num=0)
                    done += n
                return g

            # ---------------- positive index -> wrap -> gather (GpSimd early)
            rp = exact_trunc_rank(up, pc)
            geb = pool.tile([128, FT], F32)
            nc.vector.tensor_tensor(geb[:], rp[:], pbf[:], ALU.is_ge)
            rpp = pool.tile([128, FT], F32)
            nc.vector.tensor_tensor(rpp[:], rp[:], geb[:], ALU.add)
            # pos_idx[q] = sel[class(P), r'(q)] as an int16 masked sum
            rpp16 = pool.tile([128, FT], I16)
            nc.vector.tensor_copy(rpp16[:], rpp[:])
            mask3 = pool.tile([128, FT, W_s], I16)
            iota_e = iota_f[:].unsqueeze(1).broadcast_to((128, FT, W_s))
            rpp_e = rpp16[:].unsqueeze(2).broadcast_to((128, FT, W_s))
            sel_e = self_f[:].unsqueeze(1).broadcast_to((128, FT, W_s))
            nc.vector.tensor_tensor(mask3[:], iota_e, rpp_e, ALU.is_equal)
            nc.vector.tensor_tensor(mask3[:], mask3[:], sel_e, ALU.mult)
            posidx = pool.tile([128, FT], I16)
            with nc.allow_low_precision(reason="one-hot int16 sum, values < 2^13"):
                nc.vector.tensor_reduce(
                    posidx[:].rearrange("p (f o) -> p f o", o=1),
                    mask3[:], mybir.AxisListType.X, ALU.add)
            # ---------------- anchors (DVE, overlaps pos gather)
            at = pool.tile([128, A_p, D], F32)
            nc.sync.dma_start(out=at[:], in_=aemb[:].rearrange("(p t) d -> p t d", p=128))
            asq = pool.tile([128, A_p, D], F32)
            nc.vector.tensor_mul(asq[:], at[:], at[:])
            nx_a = pool.tile([128, A_p], F32)
            nc.vector.tensor_reduce(nx_a[:], asq[:], mybir.AxisListType.X, ALU.add)
            nx = nx_a[:].unsqueeze(2).broadcast_to((128, A_p, 5))  # 3D view
            a_exp = at[:].unsqueeze(2).broadcast_to((128, A_p, 5, D))

            def q3(t):  # [128, FT] tile -> [128, A_p, 5] view
                return t[:].rearrange("p (t k) -> p t k", t=A_p)

            # ---------------- negative index (DVE, overlaps pos gather)
            rn = exact_trunc_rank(un, ngc)
            tcnt = pool.tile([128, FT], F32)
            scratch = pool.tile([128, W_s], F32)
            for col in range(FT):
                nc.vector.tensor_scalar(
                    scratch[:], gpp[:], rn[:, col:col + 1], None, ALU.is_le,
                    ALU.add, accum_out=tcnt[:, col:col + 1])
            negidx_f = pool.tile([128, FT], F32)
            nc.vector.tensor_tensor(negidx_f[:], rn[:], tcnt[:], ALU.add)
            negidx = pool.tile([128, FT], I16)
            nc.vector.tensor_copy(negidx[:], negidx_f[:])
            poswr = to_wrapped(posidx, "pw")
            posg = gather_rows(poswr, "posg")

            negwr = to_wrapped(negidx, "nw")
            negg = gather_rows(negwr, "negg")

            if debug_outs:
                pf = pool.tile([128, FT], F32, tag="dbgc")
                nc.vector.tensor_copy(pf[:], posidx[:])
                nc.sync.dma_start(out=dbg_pos[:], in_=pf[:])
                nf = pool.tile([128, FT], F32, tag="dbgc2")
                nc.vector.tensor_copy(nf[:], negidx[:])
                nc.sync.dma_start(out=dbg_neg[:], in_=nf[:])

            # ---------------- Poincare distance per set (dist-pos overlaps
            # the neg gather descriptor generation)
            prod = pool.tile([128, A_p, 5, D], F32, tag="prod")
            dotv = pool.tile([128, FT], F32, tag="dotv")
            nyv = pool.tile([128, FT], F32, tag="nyv")

            def distances(g_t, name):
                g3 = g_t[:].rearrange("p (t k) d -> p t k d", t=A_p)
                nc.vector.tensor_mul(prod[:], g3, g3)
                nc.vector.tensor_reduce(
                    nyv[:].rearrange("p (t k) -> p t k", t=A_p), prod[:],
                    mybir.AxisListType.X, ALU.add)
                ny_t = nyv
                nc.vector.tensor_mul(prod[:], g3, a_exp)
                nc.vector.tensor_reduce(
                    dotv[:].rearrange("p (t k) -> p t k", t=A_p), prod[:],
                    mybir.AxisListType.X, ALU.add)
                sq = pool.tile([128, FT], F32, tag=name + "sq")
                nc.vector.tensor_tensor(q3(sq), nx, q3(ny_t), ALU.add)
                m2d = pool.tile([128, FT], F32, tag="m2d")
                nc.vector.tensor_scalar(m2d[:], dotv[:], -2.0, None, ALU.mult)
                nc.vector.tensor_tensor(sq[:], sq[:], m2d[:], ALU.add)
                onx = pool.tile([128, FT], F32, tag="onx")
                nc.vector.tensor_scalar(q3(onx), nx, -1.0, 1.0, ALU.mult, ALU.add)
                ony = pool.tile([128, FT], F32, tag="ony")
                nc.vector.tensor_scalar(ony[:], ny_t[:], -1.0, 1.0, ALU.mult, ALU.add)
                den = pool.tile([128, FT], F32, tag="den")
                nc.vector.tensor_mul(den[:], onx[:], ony[:])
                nc.vector.tensor_scalar_max(den[:], den[:], EPS)
                rec = pool.tile([128, FT], F32, tag="rec")
                nc.vector.reciprocal(rec[:], den[:])
                arg = pool.tile([128, FT], F32, tag=name + "arg")
                nc.vector.tensor_mul(arg[:], sq[:], rec[:])
                nc.vector.tensor_scalar(arg[:], arg[:], 2.0, 1.0, ALU.mult, ALU.add)
                nc.vector.tensor_scalar_max(arg[:], arg[:], 1.0 + EPS)
                # arccosh(x) = ln(x + sqrt(x^2 - 1))
                s1 = pool.tile([128, FT], F32, tag="acs1")
                nc.scalar.activation(s1[:], arg[:], ACTF.Square, bias=zerob[:])
                nc.scalar.activation(s1[:], s1[:], ACTF.Sqrt, bias=negone[:])
                nc.vector.tensor_tensor(s1[:], s1[:], arg[:], ALU.add)
                dd = pool.tile([128, FT], F32, tag=name + "d")
                nc.scalar.activation(dd[:], s1[:], ACTF.Ln, bias=zerob[:])
                return dd

            d_pos = distances(posg, "dp")
            d_neg = distances(negg, "dn")
            if debug_outs:
                nc.sync.dma_start(out=dbg_dp[:], in_=d_pos[:])
                nc.sync.dma_start(out=dbg_dn[:], in_=d_neg[:])

            # ---------------- triplet + masked partial sums
            anrm = pool.tile([128, A_p], F32)
            nc.scalar.activation(anrm[:], nx_a[:], ACTF.Sqrt, bias=zerob[:])
            marg = pool.tile([128, A_p], F32)
            nc.vector.tensor_scalar(marg[:], anrm[:], 2.0 * MARGIN, MARGIN,
                                    ALU.mult, ALU.add)
            marg_exp = marg[:].unsqueeze(2).broadcast_to((128, A_p, 5))
            trip = pool.tile([128, FT], F32)
            nc.vector.tensor_tensor(trip[:], d_pos[:], d_neg[:], ALU.subtract)
            nc.vector.tensor_tensor(q3(trip), q3(trip), marg_exp, ALU.add)
            nc.vector.tensor_scalar_max(trip[:], trip[:], 0.0)
            losses = pool.tile([128, FT], F32)
            nc.vector.tensor_mul(losses[:], trip[:], vld[:])
            act = pool.tile([128, FT], F32)
            nc.vector.tensor_scalar(act[:], trip[:], 0.0, None, ALU.is_gt)
            nc.vector.tensor_mul(act[:], act[:], vld[:])

            part = pool.tile([128, 4], F32)
            nc.vector.tensor_reduce(part[:, 0:1], losses[:], mybir.AxisListType.X, ALU.add)
            nc.vector.tensor_reduce(part[:, 1:2], act[:], mybir.AxisListType.X, ALU.add)
            nc.vector.tensor_reduce(part[:, 2:3], vld[:], mybir.AxisListType.X, ALU.add)
            nc.vector.memset(part[:, 3:4], 0.0)
            psum1 = pool.tile([1, 4], F32)
            nc.gpsimd.tensor_reduce(psum1[:], part[:], mybir.AxisListType.C, ALU.add)

            # ---------------- AllReduce partials
            ar_in = dram.tile([1, 4], F32)
            nc.sync.dma_start(out=ar_in[:], in_=psum1[:])
            ar_out = dram.tile([1, 4], F32)
            nc.gpsimd.collective_compute(
                "AllReduce", ALU.add, replica_groups=RG,
                ins=[ar_in.opt()], outs=[ar_out.opt()])
            tot = pool.tile([1, 4], F32)
            nc.sync.dma_start(out=tot[:], in_=ar_out[:])

            # ---------------- finalize: [loss, num_active, total, ratio]
            den4 = pool.tile([1, 1], F32)
            nc.vector.tensor_scalar_max(den4[:], tot[:, 2:3], 1.0)
            rec4 = pool.tile([1, 1], F32)
            nc.vector.reciprocal(rec4[:], den4[:])
            res = pool.tile([1, 4], F32)
            nc.vector.tensor_scalar(res[:, 0:1], tot[:, 0:1], rec4[:], None, ALU.mult)
            nc.vector.tensor_copy(res[:, 1:2], tot[:, 1:2])
            nc.vector.tensor_copy(res[:, 2:3], tot[:, 2:3])
            nc.vector.tensor_scalar(res[:, 3:4], tot[:, 1:2], rec4[:], None, ALU.mult)
            nc.sync.dma_start(out=out[:], in_=res[:])

    nc.finalize()  # run bacc compile (regalloc etc.) before PJRT serialization
    return nc


# ----------------------------------------------------------------------------
# entry point
# ----------------------------------------------------------------------------

_CACHE = {}


def _get_nc(A_p, FT, W_s, debug_outs):
    key = (A_p, FT, W_s, debug_outs)
    if key not in _CACHE:
        _CACHE[key] = build(A_p, FT, W_s, debug_outs)
    return _CACHE[key]


def run(inputs, debug_outs=False, trace=False):
    emb = np.ascontiguousarray(np.asarray(inputs["embeddings"], dtype=np.float32))
    labels = inputs["labels"]
    cores, A_p, FT, W_s = host_prep(labels)
    nc = _get_nc(A_p, FT, W_s, debug_outs)

    in_maps = []
    for i in range(NCORES):
        c = cores[i]
        arows = emb[c["aidx"].astype(np.int64)] * c["amask"][:, None]
        in_maps.append({
            "emb_full": emb,
            "emb_slice": np.ascontiguousarray(emb[i * (B // NCORES):(i + 1) * (B // NCORES)]),
            "aemb": np.ascontiguousarray(arows.astype(np.float32)),
            "u_pos": c["u_pos"], "u_neg": c["u_neg"],
            "poscnt": c["poscnt"], "negcnt": c["negcnt"],
            "p_bf": c["p_bf"], "valid": c["valid"],
            "sel_pp": c["sel_pp"], "g_pp": c["g_pp"],
            "sel_f": c["sel_f"], "iota_f": c["iota_f"],
        })

    res = bass_utils.run_bass_kernel_spmd(
        nc, in_maps, core_ids=list(range(NCORES)), trace=trace)
    return res, cores, A_p, FT


def kernel(**inputs):
    res, _, _, _ = run(inputs, debug_outs=False, trace=False)
    o = np.asarray(res.results[0]["out"]).reshape(4)
    loss = np.float32(o[0])
    num_active = np.int32(round(float(o[1])))
    total = np.int32(round(float(o[2])))
    ratio = np.float32(o[3])
    return loss, num_active, total, ratio


# revision 32
# speedup vs baseline: 1.3620x; 1.0808x over previous
"""AdaptiveHyperbolicTripletLoss on 8 TRN2 NeuronCores (Bass/Tile).

Strategy (class-sharded data parallel):
  - 64 label classes; core i owns classes [8i, 8i+8). Anchors are grouped by
    class onto partitions: each class occupies ceil(m_c/A_p) whole partitions
    (A_p anchor slots per partition), so every SBUF partition holds anchors of
    exactly one class. Host computes the sharding permutation + its direct
    byproducts (per-class member tables, class sizes, rank-in-class) and the
    input-independent sampling uniforms (fixed jax PRNG key 42).
  - Device computes: sampling ranks from the uniforms (exact trunc semantics),
    positive index via per-partition table gather (gpsimd indirect_copy),
    negative index via monotone member-table counting (tensor_scalar+accum),
    embedding row gathers (gpsimd dma_gather from DRAM), row norms (computed
    locally + AllGather), Poincare distances, adaptive-margin triplet loss,
    masked reduction, and a final AllReduce of the partial sums.

Query layout per core: [128 partitions, F_tot = 5*A_p columns], query
(P, F=jj*5+k) <-> (anchor slot jj of partition P, triplet k). Gather slot
order n = F*128 + P, so dma_gather output [p, i, :] is directly [P, F] aligned
and the wrapped index position (n%16, n//16) = (P%16, 8F + P//16) is affine.
"""

import math
import numpy as np

import jax

_CPU = jax.devices("cpu")[0]

from concourse import bass, bacc, tile, mybir
from concourse import bass_utils

B, D, NCLS, K = 8192, 128, 64, 5
NCORES = 8
CLS_PER_CORE = NCLS // NCORES
MARGIN, BF, EPS = 1.0, 2.0, 1e-7
BIG = 20000.0
F32 = mybir.dt.float32
BF16 = mybir.dt.bfloat16
I16 = mybir.dt.int16
U16 = mybir.dt.uint16
I32 = mybir.dt.int32
ALU = mybir.AluOpType
ACTF = mybir.ActivationFunctionType


# ----------------------------------------------------------------------------
# host-side sharding prep
# ----------------------------------------------------------------------------

def _pick_layout(cnt):
    """Smallest A_p >= 9 such that every core's classes fit in 128 partitions."""
    for A_p in range(9, 65):
        ok = True
        for i in range(NCORES):
            parts = sum(int(math.ceil(max(int(cnt[c]), 1) / A_p))
                        for c in range(i * CLS_PER_CORE, (i + 1) * CLS_PER_CORE))
            if parts > 128:
                ok = False
                break
        if ok:
            return A_p
    raise ValueError("no layout fits")


def host_prep(labels_np):
    labels = np.asarray(labels_np).astype(np.int64).ravel()
    assert labels.shape[0] == B
    cnt = np.bincount(labels, minlength=NCLS)
    A_p = _pick_layout(cnt)
    FT = 5 * A_p
    W_s = int(np.ceil((max(int(cnt.max()), 1) + 1) / 8.0)) * 8  # member-table width

    skey = jax.random.key(42)
    kp, kn = jax.random.split(skey)
    with jax.default_device(_CPU):
        u_p = np.asarray(jax.random.uniform(kp, (B, K)), dtype=np.float32)
        u_n = np.asarray(jax.random.uniform(kn, (B, K)), dtype=np.float32)

    sel = [np.where(labels == c)[0] for c in range(NCLS)]

    cores = []
    for i in range(NCORES):
        u_pos = np.zeros((128, FT), np.float32)
        u_neg = np.zeros((128, FT), np.float32)
        poscnt = np.ones((128, FT), np.float32)
        negcnt = np.ones((128, FT), np.float32)
        p_bf = np.zeros((128, FT), np.float32)
        valid = np.zeros((128, FT), np.float32)
        sel_pp = np.zeros((128, W_s), np.int16)
        g_pp = np.full((128, W_s), BIG, np.float32)
        aidx = np.zeros(128 * A_p, np.int64)
        amask = np.zeros(128 * A_p, np.float32)
        cursor = 0
        for cl in range(CLS_PER_CORE):
            c = i * CLS_PER_CORE + cl
            mem = sel[c]
            m = len(mem)
            nparts = int(math.ceil(max(m, 1) / A_p))
            prows = slice(cursor, cursor + nparts)
            if m > 0:
                sel_pp[prows, :m] = mem.astype(np.int16)[None, :]
                g_pp[prows, :m] = (mem - np.arange(m)).astype(np.float32)[None, :]
            ok = 1.0 if (2 <= m < B) else 0.0
            for s in range(m):
                P = cursor + s // A_p
                jj = s % A_p
                aidx[P * A_p + jj] = mem[s]
                amask[P * A_p + jj] = 1.0
                F0 = jj * 5
                u_pos[P, F0:F0 + 5] = u_p[mem[s]]
                u_neg[P, F0:F0 + 5] = u_n[mem[s]]
                poscnt[P, F0:F0 + 5] = m - 1 if m > 1 else 1
                negcnt[P, F0:F0 + 5] = B - m
                p_bf[P, F0:F0 + 5] = s
                valid[P, F0:F0 + 5] = ok
            cursor += nparts
        assert cursor <= 128
        cores.append(dict(u_pos=u_pos, u_neg=u_neg, poscnt=poscnt, negcnt=negcnt,
                          p_bf=p_bf, valid=valid, sel_pp=sel_pp, g_pp=g_pp,
                          sel_f=sel_pp.copy(),
                          iota_f=np.tile(np.arange(W_s, dtype=np.int16), (128, 1)),
                          aidx=aidx, amask=amask))
    return cores, A_p, FT, W_s


# ----------------------------------------------------------------------------
# device program
# ----------------------------------------------------------------------------

def build(A_p, FT, W_s, debug_outs=False):
    nc = bacc.Bacc("TRN2", target_bir_lowering=False, debug=False,
                   num_devices=NCORES)
    emb_full = nc.declare_dram_parameter("emb_full", [B, D], F32, isOutput=False)
    emb_slice = nc.declare_dram_parameter("emb_slice", [B // NCORES, D], F32, isOutput=False)
    aemb = nc.declare_dram_parameter("aemb", [128 * A_p, D], F32, isOutput=False)
    d_u_pos = nc.declare_dram_parameter("u_pos", [128, FT], F32, isOutput=False)
    d_u_neg = nc.declare_dram_parameter("u_neg", [128, FT], F32, isOutput=False)
    d_poscnt = nc.declare_dram_parameter("poscnt", [128, FT], F32, isOutput=False)
    d_negcnt = nc.declare_dram_parameter("negcnt", [128, FT], F32, isOutput=False)
    d_p_bf = nc.declare_dram_parameter("p_bf", [128, FT], F32, isOutput=False)
    d_valid = nc.declare_dram_parameter("valid", [128, FT], F32, isOutput=False)
    d_sel_pp = nc.declare_dram_parameter("sel_pp", [128, W_s], I16, isOutput=False)
    d_g_pp = nc.declare_dram_parameter("g_pp", [128, W_s], F32, isOutput=False)
    d_sel_f = nc.declare_dram_parameter("sel_f", [128, W_s], I16, isOutput=False)
    d_iota = nc.declare_dram_parameter("iota_f", [128, W_s], I16, isOutput=False)
    out = nc.declare_dram_parameter("out", [1, 4], F32, isOutput=True)
    if debug_outs:
        dbg_pos = nc.declare_dram_parameter("dbg_pos", [128, FT], F32, isOutput=True)
        dbg_neg = nc.declare_dram_parameter("dbg_neg", [128, FT], F32, isOutput=True)
        dbg_dp = nc.declare_dram_parameter("dbg_dp", [128, FT], F32, isOutput=True)
        dbg_dn = nc.declare_dram_parameter("dbg_dn", [128, FT], F32, isOutput=True)

    RG = [list(range(NCORES))]
    NV = 16 * FT  # indirect_copy valid indices per 16-partition group

    with tile.TileContext(nc) as tc:
        with tc.tile_pool(name="main", bufs=1) as pool, \
             tc.tile_pool(name="dram", bufs=1, space="DRAM") as dram:

            # ---------------- load per-query constants & tables
            up = pool.tile([128, FT], F32); nc.sync.dma_start(out=up[:], in_=d_u_pos[:])
            un = pool.tile([128, FT], F32); nc.sync.dma_start(out=un[:], in_=d_u_neg[:])
            pc = pool.tile([128, FT], F32); nc.sync.dma_start(out=pc[:], in_=d_poscnt[:])
            ngc = pool.tile([128, FT], F32); nc.sync.dma_start(out=ngc[:], in_=d_negcnt[:])
            pbf = pool.tile([128, FT], F32); nc.sync.dma_start(out=pbf[:], in_=d_p_bf[:])
            vld = pool.tile([128, FT], F32); nc.sync.dma_start(out=vld[:], in_=d_valid[:])
            gpp = pool.tile([128, W_s], F32); nc.sync.dma_start(out=gpp[:], in_=d_g_pp[:])
            self_f = pool.tile([128, W_s], I16); nc.sync.dma_start(out=self_f[:], in_=d_sel_f[:])
            iota_f = pool.tile([128, W_s], I16); nc.sync.dma_start(out=iota_f[:], in_=d_iota[:])

            def exact_trunc_rank(u, cnt_t):
                """r = min(trunc(u*cnt), max(cnt-1,0)) with rounding-mode-proof trunc."""
                x = pool.tile([128, FT], F32, tag="rk_x")
                nc.vector.tensor_mul(x[:], u[:], cnt_t[:])
                ti = pool.tile([128, FT], I32, tag="rk_ti")
                nc.vector.tensor_copy(ti[:], x[:])
                tf = pool.tile([128, FT], F32, tag="rk_tf")
                nc.vector.tensor_copy(tf[:], ti[:])
                fx = pool.tile([128, FT], F32, tag="rk_fx")
                nc.vector.tensor_tensor(fx[:], tf[:], x[:], ALU.is_gt)
                r = pool.tile([128, FT], F32, tag="rk_r")
                nc.vector.tensor_tensor(r[:], tf[:], fx[:], ALU.subtract)
                cap = pool.tile([128, FT], F32, tag="rk_cap")
                nc.vector.tensor_scalar(cap[:], cnt_t[:], 1.0, None, ALU.subtract)
                nc.vector.tensor_scalar_max(cap[:], cap[:], 0.0)
                nc.vector.tensor_tensor(r[:], r[:], cap[:], ALU.min)
                return r

            # constants + ACT table warmup (tables load during idle prologue)
            negone = pool.tile([128, 1], F32, tag="negone")
            nc.vector.memset(negone[:], -1.0)
            zerob = pool.tile([128, 1], F32, tag="zerob")
            nc.vector.memset(zerob[:], 0.0)
            warm = pool.tile([128, 1], F32, tag="warm")
            nc.scalar.activation(warm[:], zerob[:], ACTF.Square, bias=zerob[:])
            nc.scalar.activation(warm[:], zerob[:], ACTF.Sqrt, bias=zerob[:])
            nc.scalar.activation(warm[:], warm[:], ACTF.Ln, bias=negone[:])

            # ---------------- shared helpers/tiles
            NI = 128 * FT

            def to_wrapped(slot_t, name):
                wrA = pool.tile([128, 8 * FT], I16, tag=name + "A")
                # wr[p0, 8F + a] = slot[16a + p0, F]; one DMA per group a
                for a in range(8):
                    eng = nc.sync if a % 2 == 0 else nc.scalar
                    eng.dma_start(
                        out=wrA[0:16].rearrange("p (f a) -> p a f", a=8)[:, a, :],
                        in_=slot_t[16 * a:16 * (a + 1), :])
                for k in [16, 32, 64]:  # replicate idx block to all 8 gpsimd cores
                    nc.sync.dma_start(out=wrA[k:2 * k, :], in_=wrA[0:k, :])
                return wrA

            def gather_rows(wr_t, name):
                # chunked: SWDGE ring holds ~1024 descriptors
                g = pool.tile([128, FT, D], F32, tag=name)
                done = 0
                while done < NI:
                    n = min(1024, NI - done)
                    nc.gpsimd.dma_gather(
                        g[:, done // 128:(done + n) // 128, :], emb_full[:],
                        wr_t[:, done // 16:(done + n) // 16], n, n, D,
                        queue_num=0)
                    done += n
                return g

            # ---------------- positive index -> wrap -> gather (GpSimd early)
            rp = exact_trunc_rank(up, pc)
            geb = pool.tile([128, FT], F32)
            nc.vector.tensor_tensor(geb[:], rp[:], pbf[:], ALU.is_ge)
            rpp = pool.tile([128, FT], F32)
            nc.vector.tensor_tensor(rpp[:], rp[:], geb[:], ALU.add)
            # pos_idx[q] = sel[class(P), r'(q)] as an int16 masked sum.
            # Split into two F-halves with separate tiles so the first gather
            # chunks launch while the second half's masked sum still computes.
            rpp16 = pool.tile([128, FT], I16)
            nc.vector.tensor_copy(rpp16[:], rpp[:])
            FH = [(0, 8 * (FT // 16)), (8 * (FT // 16), FT)]  # chunk-aligned split

            def pos_piece(f0, f1, name):
                w = f1 - f0
                m = pool.tile([128, w, W_s], I16, tag=name + "m")
                iota_e = iota_f[:].unsqueeze(1).broadcast_to((128, w, W_s))
                rpp_e = rpp16[:, f0:f1].unsqueeze(2).broadcast_to((128, w, W_s))
                sel_e = self_f[:].unsqueeze(1).broadcast_to((128, w, W_s))
                nc.vector.tensor_tensor(m[:], iota_e, rpp_e, ALU.is_equal)
                nc.vector.tensor_tensor(m[:], m[:], sel_e, ALU.mult)
                pi = pool.tile([128, w], I16, tag=name + "i")
                with nc.allow_low_precision(reason="one-hot int16 sum, < 2^13"):
                    nc.vector.tensor_reduce(
                        pi[:].rearrange("p (f o) -> p f o", o=1),
                        m[:], mybir.AxisListType.X, ALU.add)
                wr = pool.tile([128, 8 * w], I16, tag=name + "w")
                for a in range(8):
                    eng = nc.sync if a % 2 == 0 else nc.scalar
                    eng.dma_start(
                        out=wr[0:16].rearrange("p (f a) -> p a f", a=8)[:, a, :],
                        in_=pi[16 * a:16 * (a + 1), :])
                for k in [16, 32, 64]:
                    nc.sync.dma_start(out=wr[k:2 * k, :], in_=wr[0:k, :])
                done = 128 * f0
                while done < 128 * f1:
                    n = min(1024, 128 * f1 - done)
                    nc.gpsimd.dma_gather(
                        posg[:, done // 128:(done + n) // 128, :], emb_full[:],
                        wr[:, (done - 128 * f0) // 16:(done + n - 128 * f0) // 16],
                        n, n, D, queue_num=0)
                    done += n
                return pi

            posg = pool.tile([128, FT, D], F32, tag="posg")
            pi1 = pos_piece(*FH[0], "ph1")
            # ---------------- anchors (DVE, overlaps pos gather)
            at = pool.tile([128, A_p, D], F32)
            nc.sync.dma_start(out=at[:], in_=aemb[:].rearrange("(p t) d -> p t d", p=128))
            asq = pool.tile([128, A_p, D], F32)
            nc.vector.tensor_mul(asq[:], at[:], at[:])
            nx_a = pool.tile([128, A_p], F32)
            nc.vector.tensor_reduce(nx_a[:], asq[:], mybir.AxisListType.X, ALU.add)
            nx = nx_a[:].unsqueeze(2).broadcast_to((128, A_p, 5))  # 3D view
            a_exp = at[:].unsqueeze(2).broadcast_to((128, A_p, 5, D))

            def q3(t):  # [128, FT] tile -> [128, A_p, 5] view
                return t[:].rearrange("p (t k) -> p t k", t=A_p)

            pi2 = pos_piece(*FH[1], "ph2")

            # ---------------- negative index (DVE, overlaps pos gather)
            rn = exact_trunc_rank(un, ngc)
            tcnt = pool.tile([128, FT], F32)
            scratch = pool.tile([128, W_s], F32)
            for col in range(FT):
                nc.vector.tensor_scalar(
                    scratch[:], gpp[:], rn[:, col:col + 1], None, ALU.is_le,
                    ALU.add, accum_out=tcnt[:, col:col + 1])
            negidx_f = pool.tile([128, FT], F32)
            nc.vector.tensor_tensor(negidx_f[:], rn[:], tcnt[:], ALU.add)
            negidx = pool.tile([128, FT], I16)
            nc.vector.tensor_copy(negidx[:], negidx_f[:])
            negwr = to_wrapped(negidx, "nw")
            negg = gather_rows(negwr, "negg")

            if debug_outs:
                pf = pool.tile([128, FT], F32, tag="dbgc")
                nc.vector.tensor_copy(pf[:, FH[0][0]:FH[0][1]], pi1[:])
                nc.vector.tensor_copy(pf[:, FH[1][0]:FH[1][1]], pi2[:])
                nc.sync.dma_start(out=dbg_pos[:], in_=pf[:])
                nf = pool.tile([128, FT], F32, tag="dbgc2")
                nc.vector.tensor_copy(nf[:], negidx[:])
                nc.sync.dma_start(out=dbg_neg[:], in_=nf[:])

            # ---------------- Poincare distance per set (dist-pos overlaps
            # the neg gather descriptor generation)
            prod = pool.tile([128, A_p, 5, D], F32, tag="prod")
            dotv = pool.tile([128, FT], F32, tag="dotv")
            nyv = pool.tile([128, FT], F32, tag="nyv")

            def distances(g_t, name):
                g3 = g_t[:].rearrange("p (t k) d -> p t k d", t=A_p)
                nc.vector.tensor_mul(prod[:], g3, g3)
                nc.vector.tensor_reduce(
                    nyv[:].rearrange("p (t k) -> p t k", t=A_p), prod[:],
                    mybir.AxisListType.X, ALU.add)
                ny_t = nyv
                nc.vector.tensor_mul(prod[:], g3, a_exp)
                nc.vector.tensor_reduce(
                    dotv[:].rearrange("p (t k) -> p t k", t=A_p), prod[:],
                    mybir.AxisListType.X, ALU.add)
                sq = pool.tile([128, FT], F32, tag=name + "sq")
                nc.vector.tensor_tensor(q3(sq), nx, q3(ny_t), ALU.add)
                m2d = pool.tile([128, FT], F32, tag="m2d")
                nc.vector.tensor_scalar(m2d[:], dotv[:], -2.0, None, ALU.mult)
                nc.vector.tensor_tensor(sq[:], sq[:], m2d[:], ALU.add)
                onx = pool.tile([128, FT], F32, tag="onx")
                nc.vector.tensor_scalar(q3(onx), nx, -1.0, 1.0, ALU.mult, ALU.add)
                ony = pool.tile([128, FT], F32, tag="ony")
                nc.vector.tensor_scalar(ony[:], ny_t[:], -1.0, 1.0, ALU.mult, ALU.add)
                den = pool.tile([128, FT], F32, tag="den")
                nc.vector.tensor_mul(den[:], onx[:], ony[:])
                nc.vector.tensor_scalar_max(den[:], den[:], EPS)
                rec = pool.tile([128, FT], F32, tag="rec")
                nc.vector.reciprocal(rec[:], den[:])
                arg = pool.tile([128, FT], F32, tag=name + "arg")
                nc.vector.tensor_mul(arg[:], sq[:], rec[:])
                nc.vector.tensor_scalar(arg[:], arg[:], 2.0, 1.0, ALU.mult, ALU.add)
                nc.vector.tensor_scalar_max(arg[:], arg[:], 1.0 + EPS)
                # arccosh(x) = ln(x + sqrt(x^2 - 1))
                s1 = pool.tile([128, FT], F32, tag="acs1")
                nc.scalar.activation(s1[:], arg[:], ACTF.Square, bias=zerob[:])
                nc.scalar.activation(s1[:], s1[:], ACTF.Sqrt, bias=negone[:])
                nc.vector.tensor_tensor(s1[:], s1[:], arg[:], ALU.add)
                dd = pool.tile([128, FT], F32, tag=name + "d")
                nc.scalar.activation(dd[:], s1[:], ACTF.Ln, bias=zerob[:])
                return dd

            d_pos = distances(posg, "dp")
            d_neg = distances(negg, "dn")
            if debug_outs:
                nc.sync.dma_start(out=dbg_dp[:], in_=d_pos[:])
                nc.sync.dma_start(out=dbg_dn[:], in_=d_neg[:])

            # ---------------- triplet + masked partial sums
            anrm = pool.tile([128, A_p], F32)
            nc.scalar.activation(anrm[:], nx_a[:], ACTF.Sqrt, bias=zerob[:])
            marg = pool.tile([128, A_p], F32)
            nc.vector.tensor_scalar(marg[:], anrm[:], 2.0 * MARGIN, MARGIN,
                                    ALU.mult, ALU.add)
            marg_exp = marg[:].unsqueeze(2).broadcast_to((128, A_p, 5))
            trip = pool.tile([128, FT], F32)
            nc.vector.tensor_tensor(trip[:], d_pos[:], d_neg[:], ALU.subtract)
            nc.vector.tensor_tensor(q3(trip), q3(trip), marg_exp, ALU.add)
            nc.vector.tensor_scalar_max(trip[:], trip[:], 0.0)
            losses = pool.tile([128, FT], F32)
            nc.vector.tensor_mul(losses[:], trip[:], vld[:])
            act = pool.tile([128, FT], F32)
            nc.vector.tensor_scalar(act[:], trip[:], 0.0, None, ALU.is_gt)
            nc.vector.tensor_mul(act[:], act[:], vld[:])

            part = pool.tile([128, 4], F32)
            nc.vector.tensor_reduce(part[:, 0:1], losses[:], mybir.AxisListType.X, ALU.add)
            nc.vector.tensor_reduce(part[:, 1:2], act[:], mybir.AxisListType.X, ALU.add)
            nc.vector.tensor_reduce(part[:, 2:3], vld[:], mybir.AxisListType.X, ALU.add)
            nc.vector.memset(part[:, 3:4], 0.0)
            psum1 = pool.tile([1, 4], F32)
            nc.gpsimd.tensor_reduce(psum1[:], part[:], mybir.AxisListType.C, ALU.add)

            # ---------------- AllReduce partials
            ar_in = dram.tile([1, 4], F32)
            nc.sync.dma_start(out=ar_in[:], in_=psum1[:])
            ar_out = dram.tile([1, 4], F32)
            nc.gpsimd.collective_compute(
                "AllReduce", ALU.add, replica_groups=RG,
                ins=[ar_in.opt()], outs=[ar_out.opt()])
            tot = pool.tile([1, 4], F32)
            nc.sync.dma_start(out=tot[:], in_=ar_out[:])

            # ---------------- finalize: [loss, num_active, total, ratio]
            den4 = pool.tile([1, 1], F32)
            nc.vector.tensor_scalar_max(den4[:], tot[:, 2:3], 1.0)
            rec4 = pool.tile([1, 1], F32)
            nc.vector.reciprocal(rec4[:], den4[:])
            res = pool.tile([1, 4], F32)
            nc.vector.tensor_scalar(res[:, 0:1], tot[:, 0:1], rec4[:], None, ALU.mult)
            nc.vector.tensor_copy(res[:, 1:2], tot[:, 1:2])
            nc.vector.tensor_copy(res[:, 2:3], tot[:, 2:3])
            nc.vector.tensor_scalar(res[:, 3:4], tot[:, 1:2], rec4[:], None, ALU.mult)
            nc.sync.dma_start(out=out[:], in_=res[:])

    nc.finalize()  # run bacc compile (regalloc etc.) before PJRT serialization
    return nc


# ----------------------------------------------------------------------------
# entry point
# ----------------------------------------------------------------------------

_CACHE = {}


def _get_nc(A_p, FT, W_s, debug_outs):
    key = (A_p, FT, W_s, debug_outs)
    if key not in _CACHE:
        _CACHE[key] = build(A_p, FT, W_s, debug_outs)
    return _CACHE[key]


def run(inputs, debug_outs=False, trace=False):
    emb = np.ascontiguousarray(np.asarray(inputs["embeddings"], dtype=np.float32))
    labels = inputs["labels"]
    cores, A_p, FT, W_s = host_prep(labels)
    nc = _get_nc(A_p, FT, W_s, debug_outs)

    in_maps = []
    for i in range(NCORES):
        c = cores[i]
        arows = emb[c["aidx"].astype(np.int64)] * c["amask"][:, None]
        in_maps.append({
            "emb_full": emb,
            "emb_slice": np.ascontiguousarray(emb[i * (B // NCORES):(i + 1) * (B // NCORES)]),
            "aemb": np.ascontiguousarray(arows.astype(np.float32)),
            "u_pos": c["u_pos"], "u_neg": c["u_neg"],
            "poscnt": c["poscnt"], "negcnt": c["negcnt"],
            "p_bf": c["p_bf"], "valid": c["valid"],
            "sel_pp": c["sel_pp"], "g_pp": c["g_pp"],
            "sel_f": c["sel_f"], "iota_f": c["iota_f"],
        })

    res = bass_utils.run_bass_kernel_spmd(
        nc, in_maps, core_ids=list(range(NCORES)), trace=trace)
    return res, cores, A_p, FT


def kernel(**inputs):
    res, _, _, _ = run(inputs, debug_outs=False, trace=False)
    o = np.asarray(res.results[0]["out"]).reshape(4)
    loss = np.float32(o[0])
    num_active = np.int32(round(float(o[1])))
    total = np.int32(round(float(o[2])))
    ratio = np.float32(o[3])
    return loss, num_active, total, ratio


# revision 33
# speedup vs baseline: 1.3740x; 1.0089x over previous
"""AdaptiveHyperbolicTripletLoss on 8 TRN2 NeuronCores (Bass/Tile).

Strategy (class-sharded data parallel):
  - 64 label classes; core i owns classes [8i, 8i+8). Anchors are grouped by
    class onto partitions: each class occupies ceil(m_c/A_p) whole partitions
    (A_p anchor slots per partition), so every SBUF partition holds anchors of
    exactly one class. Host computes the sharding permutation + its direct
    byproducts (per-class member tables, class sizes, rank-in-class) and the
    input-independent sampling uniforms (fixed jax PRNG key 42).
  - Device computes: sampling ranks from the uniforms (exact trunc semantics),
    positive index via per-partition table gather (gpsimd indirect_copy),
    negative index via monotone member-table counting (tensor_scalar+accum),
    embedding row gathers (gpsimd dma_gather from DRAM), row norms (computed
    locally + AllGather), Poincare distances, adaptive-margin triplet loss,
    masked reduction, and a final AllReduce of the partial sums.

Query layout per core: [128 partitions, F_tot = 5*A_p columns], query
(P, F=jj*5+k) <-> (anchor slot jj of partition P, triplet k). Gather slot
order n = F*128 + P, so dma_gather output [p, i, :] is directly [P, F] aligned
and the wrapped index position (n%16, n//16) = (P%16, 8F + P//16) is affine.
"""

import math
import numpy as np

import jax

_CPU = jax.devices("cpu")[0]

from concourse import bass, bacc, tile, mybir
from concourse import bass_utils

B, D, NCLS, K = 8192, 128, 64, 5
NCORES = 8
CLS_PER_CORE = NCLS // NCORES
MARGIN, BF, EPS = 1.0, 2.0, 1e-7
BIG = 20000.0
F32 = mybir.dt.float32
BF16 = mybir.dt.bfloat16
I16 = mybir.dt.int16
U16 = mybir.dt.uint16
I32 = mybir.dt.int32
ALU = mybir.AluOpType
ACTF = mybir.ActivationFunctionType


# ----------------------------------------------------------------------------
# host-side sharding prep
# ----------------------------------------------------------------------------

def _pick_layout(cnt):
    """Smallest A_p >= 9 such that every core's classes fit in 128 partitions."""
    for A_p in range(9, 65):
        ok = True
        for i in range(NCORES):
            parts = sum(int(math.ceil(max(int(cnt[c]), 1) / A_p))
                        for c in range(i * CLS_PER_CORE, (i + 1) * CLS_PER_CORE))
            if parts > 128:
                ok = False
                break
        if ok:
            return A_p
    raise ValueError("no layout fits")


def host_prep(labels_np):
    labels = np.asarray(labels_np).astype(np.int64).ravel()
    assert labels.shape[0] == B
    cnt = np.bincount(labels, minlength=NCLS)
    A_p = _pick_layout(cnt)
    FT = 5 * A_p
    W_s = int(np.ceil((max(int(cnt.max()), 1) + 1) / 8.0)) * 8  # member-table width

    skey = jax.random.key(42)
    kp, kn = jax.random.split(skey)
    with jax.default_device(_CPU):
        u_p = np.asarray(jax.random.uniform(kp, (B, K)), dtype=np.float32)
        u_n = np.asarray(jax.random.uniform(kn, (B, K)), dtype=np.float32)

    sel = [np.where(labels == c)[0] for c in range(NCLS)]

    cores = []
    for i in range(NCORES):
        u_pos = np.zeros((128, FT), np.float32)
        u_neg = np.zeros((128, FT), np.float32)
        poscnt = np.ones((128, FT), np.float32)
        negcnt = np.ones((128, FT), np.float32)
        p_bf = np.zeros((128, FT), np.float32)
        valid = np.zeros((128, FT), np.float32)
        sel_pp = np.zeros((128, W_s), np.int16)
        g_pp = np.full((128, W_s), BIG, np.float32)
        aidx = np.zeros(128 * A_p, np.int64)
        amask = np.zeros(128 * A_p, np.float32)
        cursor = 0
        for cl in range(CLS_PER_CORE):
            c = i * CLS_PER_CORE + cl
            mem = sel[c]
            m = len(mem)
            nparts = int(math.ceil(max(m, 1) / A_p))
            prows = slice(cursor, cursor + nparts)
            if m > 0:
                sel_pp[prows, :m] = mem.astype(np.int16)[None, :]
                g_pp[prows, :m] = (mem - np.arange(m)).astype(np.float32)[None, :]
            ok = 1.0 if (2 <= m < B) else 0.0
            for s in range(m):
                P = cursor + s // A_p
                jj = s % A_p
                aidx[P * A_p + jj] = mem[s]
                amask[P * A_p + jj] = 1.0
                F0 = jj * 5
                u_pos[P, F0:F0 + 5] = u_p[mem[s]]
                u_neg[P, F0:F0 + 5] = u_n[mem[s]]
                poscnt[P, F0:F0 + 5] = m - 1 if m > 1 else 1
                negcnt[P, F0:F0 + 5] = B - m
                p_bf[P, F0:F0 + 5] = s
                valid[P, F0:F0 + 5] = ok
            cursor += nparts
        assert cursor <= 128
        cores.append(dict(u_pos=u_pos, u_neg=u_neg, poscnt=poscnt, negcnt=negcnt,
                          p_bf=p_bf, valid=valid, sel_pp=sel_pp, g_pp=g_pp,
                          sel_f=sel_pp.copy(),
                          iota_f=np.tile(np.arange(W_s, dtype=np.int16), (128, 1)),
                          aidx=aidx, amask=amask))
    return cores, A_p, FT, W_s


# ----------------------------------------------------------------------------
# device program
# ----------------------------------------------------------------------------

def build(A_p, FT, W_s, debug_outs=False):
    nc = bacc.Bacc("TRN2", target_bir_lowering=False, debug=False,
                   num_devices=NCORES)
    emb_full = nc.declare_dram_parameter("emb_full", [B, D], F32, isOutput=False)
    emb_slice = nc.declare_dram_parameter("emb_slice", [B // NCORES, D], F32, isOutput=False)
    aemb = nc.declare_dram_parameter("aemb", [128 * A_p, D], F32, isOutput=False)
    d_u_pos = nc.declare_dram_parameter("u_pos", [128, FT], F32, isOutput=False)
    d_u_neg = nc.declare_dram_parameter("u_neg", [128, FT], F32, isOutput=False)
    d_poscnt = nc.declare_dram_parameter("poscnt", [128, FT], F32, isOutput=False)
    d_negcnt = nc.declare_dram_parameter("negcnt", [128, FT], F32, isOutput=False)
    d_p_bf = nc.declare_dram_parameter("p_bf", [128, FT], F32, isOutput=False)
    d_valid = nc.declare_dram_parameter("valid", [128, FT], F32, isOutput=False)
    d_sel_pp = nc.declare_dram_parameter("sel_pp", [128, W_s], I16, isOutput=False)
    d_g_pp = nc.declare_dram_parameter("g_pp", [128, W_s], F32, isOutput=False)
    d_sel_f = nc.declare_dram_parameter("sel_f", [128, W_s], I16, isOutput=False)
    d_iota = nc.declare_dram_parameter("iota_f", [128, W_s], I16, isOutput=False)
    out = nc.declare_dram_parameter("out", [1, 4], F32, isOutput=True)
    if debug_outs:
        dbg_pos = nc.declare_dram_parameter("dbg_pos", [128, FT], F32, isOutput=True)
        dbg_neg = nc.declare_dram_parameter("dbg_neg", [128, FT], F32, isOutput=True)
        dbg_dp = nc.declare_dram_parameter("dbg_dp", [128, FT], F32, isOutput=True)
        dbg_dn = nc.declare_dram_parameter("dbg_dn", [128, FT], F32, isOutput=True)

    RG = [list(range(NCORES))]
    NV = 16 * FT  # indirect_copy valid indices per 16-partition group

    with tile.TileContext(nc) as tc:
        with tc.tile_pool(name="main", bufs=1) as pool, \
             tc.tile_pool(name="dram", bufs=1, space="DRAM") as dram:

            # ---------------- load per-query constants & tables
            up = pool.tile([128, FT], F32); nc.sync.dma_start(out=up[:], in_=d_u_pos[:])
            un = pool.tile([128, FT], F32); nc.sync.dma_start(out=un[:], in_=d_u_neg[:])
            pc = pool.tile([128, FT], F32); nc.sync.dma_start(out=pc[:], in_=d_poscnt[:])
            ngc = pool.tile([128, FT], F32); nc.sync.dma_start(out=ngc[:], in_=d_negcnt[:])
            pbf = pool.tile([128, FT], F32); nc.sync.dma_start(out=pbf[:], in_=d_p_bf[:])
            vld = pool.tile([128, FT], F32); nc.sync.dma_start(out=vld[:], in_=d_valid[:])
            gpp = pool.tile([128, W_s], F32); nc.sync.dma_start(out=gpp[:], in_=d_g_pp[:])
            self_f = pool.tile([128, W_s], I16); nc.sync.dma_start(out=self_f[:], in_=d_sel_f[:])
            iota_f = pool.tile([128, W_s], I16); nc.sync.dma_start(out=iota_f[:], in_=d_iota[:])

            def exact_trunc_rank(u, cnt_t):
                """r = min(trunc(u*cnt), max(cnt-1,0)) with rounding-mode-proof trunc."""
                x = pool.tile([128, FT], F32, tag="rk_x")
                nc.vector.tensor_mul(x[:], u[:], cnt_t[:])
                ti = pool.tile([128, FT], I32, tag="rk_ti")
                nc.vector.tensor_copy(ti[:], x[:])
                tf = pool.tile([128, FT], F32, tag="rk_tf")
                nc.vector.tensor_copy(tf[:], ti[:])
                fx = pool.tile([128, FT], F32, tag="rk_fx")
                nc.vector.tensor_tensor(fx[:], tf[:], x[:], ALU.is_gt)
                r = pool.tile([128, FT], F32, tag="rk_r")
                nc.vector.tensor_tensor(r[:], tf[:], fx[:], ALU.subtract)
                cap = pool.tile([128, FT], F32, tag="rk_cap")
                nc.vector.tensor_scalar(cap[:], cnt_t[:], 1.0, None, ALU.subtract)
                nc.vector.tensor_scalar_max(cap[:], cap[:], 0.0)
                nc.vector.tensor_tensor(r[:], r[:], cap[:], ALU.min)
                return r

            # constants + ACT table warmup (tables load during idle prologue)
            negone = pool.tile([128, 1], F32, tag="negone")
            nc.vector.memset(negone[:], -1.0)
            zerob = pool.tile([128, 1], F32, tag="zerob")
            nc.vector.memset(zerob[:], 0.0)
            warm = pool.tile([128, 1], F32, tag="warm")
            nc.scalar.activation(warm[:], zerob[:], ACTF.Square, bias=zerob[:])
            nc.scalar.activation(warm[:], zerob[:], ACTF.Sqrt, bias=zerob[:])
            nc.scalar.activation(warm[:], warm[:], ACTF.Ln, bias=negone[:])

            # ---------------- shared helpers/tiles
            NI = 128 * FT

            def to_wrapped(slot_t, name):
                wrA = pool.tile([128, 8 * FT], I16, tag=name + "A")
                # wr[p0, 8F + a] = slot[16a + p0, F]; one DMA per group a
                for a in range(8):
                    eng = nc.sync if a % 2 == 0 else nc.scalar
                    eng.dma_start(
                        out=wrA[0:16].rearrange("p (f a) -> p a f", a=8)[:, a, :],
                        in_=slot_t[16 * a:16 * (a + 1), :])
                for k in [16, 32, 64]:  # replicate idx block to all 8 gpsimd cores
                    nc.sync.dma_start(out=wrA[k:2 * k, :], in_=wrA[0:k, :])
                return wrA

            def gather_rows(wr_t, name):
                # chunked: SWDGE ring holds ~1024 descriptors
                g = pool.tile([128, FT, D], F32, tag=name)
                done = 0
                while done < NI:
                    n = min(1024, NI - done)
                    nc.gpsimd.dma_gather(
                        g[:, done // 128:(done + n) // 128, :], emb_full[:],
                        wr_t[:, done // 16:(done + n) // 16], n, n, D,
                        queue_num=0)
                    done += n
                return g

            # ---------------- positive index -> wrap -> gather (GpSimd early)
            rp = exact_trunc_rank(up, pc)
            geb = pool.tile([128, FT], F32)
            nc.vector.tensor_tensor(geb[:], rp[:], pbf[:], ALU.is_ge)
            rpp = pool.tile([128, FT], F32)
            nc.vector.tensor_tensor(rpp[:], rp[:], geb[:], ALU.add)
            # pos_idx[q] = sel[class(P), r'(q)] as an int16 masked sum.
            # Split into two F-halves with separate tiles so the first gather
            # chunks launch while the second half's masked sum still computes.
            rpp16 = pool.tile([128, FT], I16)
            nc.vector.tensor_copy(rpp16[:], rpp[:])
            FH = [(0, 8), (8, 24), (24, FT)]  # chunk-aligned pieces

            def pos_piece(f0, f1, name):
                w = f1 - f0
                m = pool.tile([128, w, W_s], I16, tag=name + "m")
                iota_e = iota_f[:].unsqueeze(1).broadcast_to((128, w, W_s))
                rpp_e = rpp16[:, f0:f1].unsqueeze(2).broadcast_to((128, w, W_s))
                sel_e = self_f[:].unsqueeze(1).broadcast_to((128, w, W_s))
                nc.vector.tensor_tensor(m[:], iota_e, rpp_e, ALU.is_equal)
                nc.vector.tensor_tensor(m[:], m[:], sel_e, ALU.mult)
                pi = pool.tile([128, w], I16, tag=name + "i")
                with nc.allow_low_precision(reason="one-hot int16 sum, < 2^13"):
                    nc.vector.tensor_reduce(
                        pi[:].rearrange("p (f o) -> p f o", o=1),
                        m[:], mybir.AxisListType.X, ALU.add)
                wr = pool.tile([128, 8 * w], I16, tag=name + "w")
                for a in range(8):
                    eng = nc.sync if a % 2 == 0 else nc.scalar
                    eng.dma_start(
                        out=wr[0:16].rearrange("p (f a) -> p a f", a=8)[:, a, :],
                        in_=pi[16 * a:16 * (a + 1), :])
                for k in [16, 32, 64]:
                    nc.sync.dma_start(out=wr[k:2 * k, :], in_=wr[0:k, :])
                done = 128 * f0
                while done < 128 * f1:
                    n = min(1024, 128 * f1 - done)
                    nc.gpsimd.dma_gather(
                        posg[:, done // 128:(done + n) // 128, :], emb_full[:],
                        wr[:, (done - 128 * f0) // 16:(done + n - 128 * f0) // 16],
                        n, n, D, queue_num=0)
                    done += n
                return pi

            posg = pool.tile([128, FT, D], F32, tag="posg")
            pi1 = pos_piece(*FH[0], "ph1")
            pi1b = pos_piece(*FH[1], "ph1b")
            # ---------------- anchors (DVE, overlaps pos gather)
            at = pool.tile([128, A_p, D], F32)
            nc.sync.dma_start(out=at[:], in_=aemb[:].rearrange("(p t) d -> p t d", p=128))
            asq = pool.tile([128, A_p, D], F32)
            nc.vector.tensor_mul(asq[:], at[:], at[:])
            nx_a = pool.tile([128, A_p], F32)
            nc.vector.tensor_reduce(nx_a[:], asq[:], mybir.AxisListType.X, ALU.add)
            nx = nx_a[:].unsqueeze(2).broadcast_to((128, A_p, 5))  # 3D view
            a_exp = at[:].unsqueeze(2).broadcast_to((128, A_p, 5, D))

            def q3(t):  # [128, FT] tile -> [128, A_p, 5] view
                return t[:].rearrange("p (t k) -> p t k", t=A_p)

            pi2 = pos_piece(*FH[2], "ph2")

            # ---------------- negative index (DVE, overlaps pos gather)
            rn = exact_trunc_rank(un, ngc)
            tcnt = pool.tile([128, FT], F32)
            scratch = pool.tile([128, W_s], F32)
            for col in range(FT):
                nc.vector.tensor_scalar(
                    scratch[:], gpp[:], rn[:, col:col + 1], None, ALU.is_le,
                    ALU.add, accum_out=tcnt[:, col:col + 1])
            negidx_f = pool.tile([128, FT], F32)
            nc.vector.tensor_tensor(negidx_f[:], rn[:], tcnt[:], ALU.add)
            negidx = pool.tile([128, FT], I16)
            nc.vector.tensor_copy(negidx[:], negidx_f[:])
            negwr = to_wrapped(negidx, "nw")
            negg = gather_rows(negwr, "negg")

            if debug_outs:
                pf = pool.tile([128, FT], F32, tag="dbgc")
                nc.vector.tensor_copy(pf[:, FH[0][0]:FH[0][1]], pi1[:])
                nc.vector.tensor_copy(pf[:, FH[1][0]:FH[1][1]], pi1b[:])
                nc.vector.tensor_copy(pf[:, FH[2][0]:FH[2][1]], pi2[:])
                nc.sync.dma_start(out=dbg_pos[:], in_=pf[:])
                nf = pool.tile([128, FT], F32, tag="dbgc2")
                nc.vector.tensor_copy(nf[:], negidx[:])
                nc.sync.dma_start(out=dbg_neg[:], in_=nf[:])

            # ---------------- Poincare distance per set (dist-pos overlaps
            # the neg gather descriptor generation)
            prod = pool.tile([128, A_p, 5, D], F32, tag="prod")
            dotv = pool.tile([128, FT], F32, tag="dotv")
            nyv = pool.tile([128, FT], F32, tag="nyv")

            def distances(g_t, name):
                g3 = g_t[:].rearrange("p (t k) d -> p t k d", t=A_p)
                nc.vector.tensor_mul(prod[:], g3, g3)
                nc.vector.tensor_reduce(
                    nyv[:].rearrange("p (t k) -> p t k", t=A_p), prod[:],
                    mybir.AxisListType.X, ALU.add)
                ny_t = nyv
                nc.vector.tensor_mul(prod[:], g3, a_exp)
                nc.vector.tensor_reduce(
                    dotv[:].rearrange("p (t k) -> p t k", t=A_p), prod[:],
                    mybir.AxisListType.X, ALU.add)
                sq = pool.tile([128, FT], F32, tag=name + "sq")
                nc.vector.tensor_tensor(q3(sq), nx, q3(ny_t), ALU.add)
                m2d = pool.tile([128, FT], F32, tag="m2d")
                nc.vector.tensor_scalar(m2d[:], dotv[:], -2.0, None, ALU.mult)
                nc.vector.tensor_tensor(sq[:], sq[:], m2d[:], ALU.add)
                onx = pool.tile([128, FT], F32, tag="onx")
                nc.vector.tensor_scalar(q3(onx), nx, -1.0, 1.0, ALU.mult, ALU.add)
                ony = pool.tile([128, FT], F32, tag="ony")
                nc.vector.tensor_scalar(ony[:], ny_t[:], -1.0, 1.0, ALU.mult, ALU.add)
                den = pool.tile([128, FT], F32, tag="den")
                nc.vector.tensor_mul(den[:], onx[:], ony[:])
                nc.vector.tensor_scalar_max(den[:], den[:], EPS)
                rec = pool.tile([128, FT], F32, tag="rec")
                nc.vector.reciprocal(rec[:], den[:])
                arg = pool.tile([128, FT], F32, tag=name + "arg")
                nc.vector.tensor_mul(arg[:], sq[:], rec[:])
                nc.vector.tensor_scalar(arg[:], arg[:], 2.0, 1.0, ALU.mult, ALU.add)
                nc.vector.tensor_scalar_max(arg[:], arg[:], 1.0 + EPS)
                # arccosh(x) = ln(x + sqrt(x^2 - 1))
                s1 = pool.tile([128, FT], F32, tag="acs1")
                nc.scalar.activation(s1[:], arg[:], ACTF.Square, bias=zerob[:])
                nc.scalar.activation(s1[:], s1[:], ACTF.Sqrt, bias=negone[:])
                nc.vector.tensor_tensor(s1[:], s1[:], arg[:], ALU.add)
                dd = pool.tile([128, FT], F32, tag=name + "d")
                nc.scalar.activation(dd[:], s1[:], ACTF.Ln, bias=zerob[:])
                return dd

            d_pos = distances(posg, "dp")
            d_neg = distances(negg, "dn")
            if debug_outs:
                nc.sync.dma_start(out=dbg_dp[:], in_=d_pos[:])
                nc.sync.dma_start(out=dbg_dn[:], in_=d_neg[:])

            # ---------------- triplet + masked partial sums
            anrm = pool.tile([128, A_p], F32)
            nc.scalar.activation(anrm[:], nx_a[:], ACTF.Sqrt, bias=zerob[:])
            marg = pool.tile([128, A_p], F32)
            nc.vector.tensor_scalar(marg[:], anrm[:], 2.0 * MARGIN, MARGIN,
                                    ALU.mult, ALU.add)
            marg_exp = marg[:].unsqueeze(2).broadcast_to((128, A_p, 5))
            trip = pool.tile([128, FT], F32)
            nc.vector.tensor_tensor(trip[:], d_pos[:], d_neg[:], ALU.subtract)
            nc.vector.tensor_tensor(q3(trip), q3(trip), marg_exp, ALU.add)
            nc.vector.tensor_scalar_max(trip[:], trip[:], 0.0)
            losses = pool.tile([128, FT], F32)
            nc.vector.tensor_mul(losses[:], trip[:], vld[:])
            act = pool.tile([128, FT], F32)
            nc.vector.tensor_scalar(act[:], trip[:], 0.0, None, ALU.is_gt)
            nc.vector.tensor_mul(act[:], act[:], vld[:])

            part = pool.tile([128, 4], F32)
            nc.vector.tensor_reduce(part[:, 0:1], losses[:], mybir.AxisListType.X, ALU.add)
            nc.vector.tensor_reduce(part[:, 1:2], act[:], mybir.AxisListType.X, ALU.add)
            nc.vector.tensor_reduce(part[:, 2:3], vld[:], mybir.AxisListType.X, ALU.add)
            nc.vector.memset(part[:, 3:4], 0.0)
            psum1 = pool.tile([1, 4], F32)
            nc.gpsimd.tensor_reduce(psum1[:], part[:], mybir.AxisListType.C, ALU.add)

            # ---------------- AllReduce partials
            ar_in = dram.tile([1, 4], F32)
            nc.sync.dma_start(out=ar_in[:], in_=psum1[:])
            ar_out = dram.tile([1, 4], F32)
            nc.gpsimd.collective_compute(
                "AllReduce", ALU.add, replica_groups=RG,
                ins=[ar_in.opt()], outs=[ar_out.opt()])
            tot = pool.tile([1, 4], F32)
            nc.sync.dma_start(out=tot[:], in_=ar_out[:])

            # ---------------- finalize: [loss, num_active, total, ratio]
            den4 = pool.tile([1, 1], F32)
            nc.vector.tensor_scalar_max(den4[:], tot[:, 2:3], 1.0)
            rec4 = pool.tile([1, 1], F32)
            nc.vector.reciprocal(rec4[:], den4[:])
            res = pool.tile([1, 4], F32)
            nc.vector.tensor_scalar(res[:, 0:1], tot[:, 0:1], rec4[:], None, ALU.mult)
            nc.vector.tensor_copy(res[:, 1:2], tot[:, 1:2])
            nc.vector.tensor_copy(res[:, 2:3], tot[:, 2:3])
            nc.vector.tensor_scalar(res[:, 3:4], tot[:, 1:2], rec4[:], None, ALU.mult)
            nc.sync.dma_start(out=out[:], in_=res[:])

    nc.finalize()  # run bacc compile (regalloc etc.) before PJRT serialization
    return nc


# ----------------------------------------------------------------------------
# entry point
# ----------------------------------------------------------------------------

_CACHE = {}


def _get_nc(A_p, FT, W_s, debug_outs):
    key = (A_p, FT, W_s, debug_outs)
    if key not in _CACHE:
        _CACHE[key] = build(A_p, FT, W_s, debug_outs)
    return _CACHE[key]


def run(inputs, debug_outs=False, trace=False):
    emb = np.ascontiguousarray(np.asarray(inputs["embeddings"], dtype=np.float32))
    labels = inputs["labels"]
    cores, A_p, FT, W_s = host_prep(labels)
    nc = _get_nc(A_p, FT, W_s, debug_outs)

    in_maps = []
    for i in range(NCORES):
        c = cores[i]
        arows = emb[c["aidx"].astype(np.int64)] * c["amask"][:, None]
        in_maps.append({
            "emb_full": emb,
            "emb_slice": np.ascontiguousarray(emb[i * (B // NCORES):(i + 1) * (B // NCORES)]),
            "aemb": np.ascontiguousarray(arows.astype(np.float32)),
            "u_pos": c["u_pos"], "u_neg": c["u_neg"],
            "poscnt": c["poscnt"], "negcnt": c["negcnt"],
            "p_bf": c["p_bf"], "valid": c["valid"],
            "sel_pp": c["sel_pp"], "g_pp": c["g_pp"],
            "sel_f": c["sel_f"], "iota_f": c["iota_f"],
        })

    res = bass_utils.run_bass_kernel_spmd(
        nc, in_maps, core_ids=list(range(NCORES)), trace=trace)
    return res, cores, A_p, FT


def kernel(**inputs):
    res, _, _, _ = run(inputs, debug_outs=False, trace=False)
    o = np.asarray(res.results[0]["out"]).reshape(4)
    loss = np.float32(o[0])
    num_active = np.int32(round(float(o[1])))
    total = np.int32(round(float(o[2])))
    ratio = np.float32(o[3])
    return loss, num_active, total, ratio
